# revision 22
# baseline (speedup 1.0000x reference)
"""MLA-style attention kernel for 8 TRN2 NeuronCores (v5).

Sharding: core c -> batch b = c//4, heads r*4..r*4+3 where r = c%4.

v5 vs v3-baseline: the AllGather is gone -- every core computes cq for
the FULL T with fp8 DoubleRow matmuls (same PE cost as the old own-chunk
bf16 cq, no collective, no dependency stall).  The score path is fully
fp8-DR (cq, kr, K up-projection contract in fp8 pairs); the value path
(ckv -> V -> PV -> W_o) stays bf16 end-to-end for accuracy.  The softmax
denominator is computed with 1-row matmuls (probability tile stationary,
a ones column moving), transposed to row form with one PE transpose, so
the old 512-row ones-matmuls disappear.  exp runs on [128,1024] score
pairs (two k-tiles per 2-bank PSUM tile) to halve ACT instruction count.
W_o tiles for the previous chunk are interleaved into the attention pair
loop so PE never waits on exp; output copies run on the Pool engine.
Rope multiplies run on 64-row two-head bands and the final sub/add write
fp8 score operands directly (no separate convert).  Q/V/K up-projection
chains are interleaved round-robin with their converts spread across
DVE/Pool/ACT so no single engine paces phase B.
"""
import math
import numpy as np
import ml_dtypes

import concourse.bass as bass
import concourse.bacc as bacc
import concourse.mybir as mybir
import concourse.tile as tile
from concourse.bass_utils import run_bass_kernel_spmd

F32 = mybir.dt.float32
BF16 = mybir.dt.bfloat16
FP8 = mybir.dt.float8e4
QKS = 16.0            # fp8 scale for q/k operands
WUS = 64.0            # fp8 scale for weights
SX = 4.0              # fp8 scale for x (score path)
Exp = mybir.ActivationFunctionType.Exp
Copy = mybir.ActivationFunctionType.Copy

B, T, C = 2, 2048, 2048
H = 16
HS = 128
NL = 512
RHD = 64
HLOC = 4              # heads per core
P = 128
NNL = NL // P         # 4 latent p-tiles
TCH = 512
NCH = T // TCH        # 4 T-chunks
NCT = C // P          # 16 contraction p-tiles over C
SCALE = 1.0 / math.sqrt(HS + RHD)
SCALE8 = SCALE / (QKS * QKS)
NEG = -1.0e30
DR = mybir.MatmulPerfMode.DoubleRow

_NC_CACHE = {}
DEN_TRICK = True


def build():
    nc = bacc.Bacc("TRN2", target_bir_lowering=False, debug=False, num_devices=8)

    xT_ext = nc.dram_tensor("xT", [C, T], BF16, kind="ExternalInput")
    x8_ext = nc.dram_tensor("x8", [C, T], FP8, kind="ExternalInput")
    wdq8_ext = nc.dram_tensor("wdq8", [C, NL], FP8, kind="ExternalInput")
    wdkvT_ext = nc.dram_tensor("wdkvT", [C, NL], BF16, kind="ExternalInput")
    wkr8_ext = nc.dram_tensor("wkr8", [C, RHD], FP8, kind="ExternalInput")
    wuqT_ext = nc.dram_tensor("wuqT", [NL, HLOC * HS], FP8, kind="ExternalInput")
    wukT_ext = nc.dram_tensor("wukT", [NL, HLOC * HS], FP8, kind="ExternalInput")
    wuvT_ext = nc.dram_tensor("wuvT", [NL, HLOC * HS], BF16, kind="ExternalInput")
    wqrT_ext = nc.dram_tensor("wqrT", [NL, HLOC * RHD], FP8, kind="ExternalInput")
    woT_ext = nc.dram_tensor("woT", [HLOC * HS, C], BF16, kind="ExternalInput")
    caT_ext = nc.dram_tensor("caT", [P, T], BF16, kind="ExternalInput")
    saT_ext = nc.dram_tensor("saT", [P, T], BF16, kind="ExternalInput")
    out_ext = nc.dram_tensor("out", [C, T], BF16, kind="ExternalOutput")
    xo8_ext = nc.dram_tensor("xo8", [C, TCH], FP8, kind="ExternalInput")
    agin_dram = nc.dram_tensor("agin", [NL, TCH], FP8)
    agout_dram = nc.dram_tensor("agout", [NCH, NL, TCH], FP8)

    ones_dram = nc.inline_tensor(np.ones((P, P), dtype=ml_dtypes.bfloat16),
                                 name="onesc")
    ident_dram = nc.inline_tensor(np.eye(P, dtype=ml_dtypes.bfloat16),
                                  name="identc")
    # boundary mask for the diagonal 128-col sub-block of S^T tiles [k, q]:
    # m2[jj, u] = 0 if u >= jj else -1e30
    m2 = np.zeros((P, P), dtype=ml_dtypes.bfloat16)
    for jj in range(P):
        m2[jj, jj:] = 1.0
    m2_dram = nc.inline_tensor(m2, name="m2c")

    with tile.TileContext(nc) as tc:
        with (
            tc.tile_pool(name="pers", bufs=1) as pers,
            tc.tile_pool(name="pbig", bufs=2, space="PSUM") as pbig,
            tc.tile_pool(name="pwk", bufs=1, space="PSUM") as pwk,
        ):
            ones = pers.tile([P, P], BF16, tag="ones", name="ones")
            zers = pers.tile([P, P], BF16, tag="zers", name="zers")
            nc.gpsimd.memset(zers[:], 0.0)
            ident = pers.tile([P, P], BF16, tag="ident", name="ident")
            m2b = pers.tile([P, P], BF16, tag="m2b", name="m2b")

            ca = pers.tile([P, T], BF16, tag="ca", name="ca")
            sa = pers.tile([P, T], BF16, tag="sa", name="sa")
            # PE p-state warmup: dummy matmuls on not-yet-loaded tiles keep the
            # tensor engine continuously busy through the initial DMA wait so
            # real matmuls start at full clock.
            for wi in range(14):
                warm = pbig.tile([P, 2 * TCH], F32, tag="big", name="big")
                nc.tensor.matmul(
                    warm[:, 0:TCH], ca[:, 0:P], sa[:, 0:TCH],
                    start=True, stop=True, skip_group_check=True,
                )

            # persistent activations
            ckv_sb = pers.tile([P, NNL * T], BF16, tag="ckv", name="ckv")
            ckv8 = pers.tile([P, NNL * T], FP8, tag="ckv8", name="ckv8")
            cq8 = pers.tile([P, NNL * T], FP8, tag="cq8", name="cq8")
            kr = pers.tile([RHD, T], BF16, tag="kr", name="kr")

            # score operands + V live in the pers pool so their memsets can
            # run at t=0 under the initial DMA window.
            qk8 = [pers.tile([P, 2 * T], FP8, tag=f"qk8{i}", name=f"qk8{i}")
                   for i in range(HLOC)]
            kk8 = [pers.tile([P, 2 * T], FP8, tag=f"kk8{i}", name=f"kk8{i}")
                   for i in range(HLOC)]
            for i in range(HLOC):
                nc.gpsimd.memset(qk8[i][RHD:P, T:2 * T], 0.0)
                nc.gpsimd.memset(kk8[i][RHD:P, T:2 * T], 0.0)
            vv = pers.tile([P, (T // P) * TCH], BF16, tag="vv", name="vv")

            # up/out-projection weights (preloaded early, used later)
            wuq_sb = pers.tile([P, NNL * HLOC * HS], FP8, tag="wuq", name="wuq")
            wuk_sb = pers.tile([P, NNL * HLOC * HS], FP8, tag="wuk", name="wuk")
            wuv_sb = pers.tile([P, NNL * HLOC * HS], BF16, tag="wuv", name="wuv")
            wqr_sb = pers.tile([P, NNL * HLOC * RHD], FP8, tag="wqr", name="wqr")
            wo_sb = pers.tile([P, HLOC * C], BF16, tag="wo", name="wo")

            # ---------------- phase A: down-projections ----------------
            with tc.tile_pool(name="pa", bufs=1) as pa:
                wdq8_sb = pa.tile([P, NCT * NL], FP8, tag="wdq", name="wdq")
                wdkv_sb = pa.tile([P, NCT * NL], BF16, tag="wdkv", name="wdkv")
                wkr8_sb = pa.tile([P, NCT * RHD], FP8, tag="wkr", name="wkr")
                xcs = [pa.tile([P, NCT * TCH], BF16, tag="xc", bufs=2,
                               name="xc") for _ in range(NCH)]
                x8s = [pa.tile([P, NCT * TCH], FP8, tag="x8c", bufs=2,
                               name="x8c") for _ in range(NCH)]

                def dma_x(ch):
                    tsl = slice(ch * TCH, (ch + 1) * TCH)
                    for st in range(4):
                        nc.sync.dma_start(
                            xcs[ch][:, st * 4 * TCH:(st + 1) * 4 * TCH].rearrange(
                                "p (a w) -> p a w", a=4),
                            xT_ext.ap()[st * 4 * P:(st + 1) * 4 * P, tsl].rearrange(
                                "(a p) w -> p a w", p=P),
                        )
                    for st in range(2):
                        nc.sync.dma_start(
                            x8s[ch][:, st * 8 * TCH:(st + 1) * 8 * TCH].rearrange(
                                "p (a w) -> p a w", a=8),
                            x8_ext.ap()[st * 8 * P:(st + 1) * 8 * P, tsl].rearrange(
                                "(a p) w -> p a w", p=P),
                        )

                # critical-path first: wdkv + chunk-0 x stripes interleaved
                for st in range(4):
                    nc.sync.dma_start(
                        wdkv_sb[:, st * 4 * NL:(st + 1) * 4 * NL].rearrange(
                            "p (a w) -> p a w", a=4),
                        wdkvT_ext.ap()[st * 4 * P:(st + 1) * 4 * P, :].rearrange(
                            "(a p) w -> p a w", p=P),
                    )
                    nc.sync.dma_start(
                        xcs[0][:, st * 4 * TCH:(st + 1) * 4 * TCH].rearrange(
                            "p (a w) -> p a w", a=4),
                        xT_ext.ap()[st * 4 * P:(st + 1) * 4 * P, 0:TCH].rearrange(
                            "(a p) w -> p a w", p=P),
                    )
                for st in range(2):
                    nc.sync.dma_start(
                        wdq8_sb[:, st * 8 * NL:(st + 1) * 8 * NL].rearrange(
                            "p (a w) -> p a w", a=8),
                        wdq8_ext.ap()[st * 8 * P:(st + 1) * 8 * P, :].rearrange(
                            "(a p) w -> p a w", p=P),
                    )
                    nc.sync.dma_start(
                        x8s[0][:, st * 8 * TCH:(st + 1) * 8 * TCH].rearrange(
                            "p (a w) -> p a w", a=8),
                        x8_ext.ap()[st * 8 * P:(st + 1) * 8 * P, 0:TCH].rearrange(
                            "(a p) w -> p a w", p=P),
                    )
                nc.sync.dma_start(
                    wkr8_sb[:].rearrange("p (a w) -> p a w", a=NCT),
                    wkr8_ext.ap().rearrange("(a p) w -> p a w", p=P),
                )
                dma_x(1)
                # non-critical loads on the Pool (SWDGE) queue
                nc.gpsimd.dma_start(out=ones[:], in_=ones_dram.ap())
                nc.gpsimd.dma_start(out=ident[:], in_=ident_dram.ap())
                nc.gpsimd.dma_start(out=m2b[:], in_=m2_dram.ap())
                nc.gpsimd.dma_start(out=ca[:], in_=caT_ext.ap())
                nc.gpsimd.dma_start(out=sa[:], in_=saT_ext.ap())
                # phase-B weights: needed only after ~70us, issue behind x
                for ext, sb in ((wuqT_ext, wuq_sb), (wqrT_ext, wqr_sb),
                                (wukT_ext, wuk_sb), (wuvT_ext, wuv_sb)):
                    nc.sync.dma_start(
                        sb[:].rearrange("p (a w) -> p a w", a=NNL),
                        ext.ap().rearrange("(a p) w -> p a w", p=P),
                    )
                nc.sync.dma_start(
                    wo_sb[:].rearrange("p (a w) -> p a w", a=HLOC),
                    woT_ext.ap().rearrange("(a p) w -> p a w", p=P),
                )

                wdq8v = wdq8_sb[:].rearrange("p (a w) -> p a w", a=NCT)
                wkr8v = wkr8_sb[:].rearrange("p (a w) -> p a w", a=NCT)

                # cq for the OWN T-chunk only (host stages xo8 per core);
                # AllGathered in fp8 across the 4-core group and consumed
                # directly by the fp8 Q up-projections.  The collective runs
                # under the whole of phase A.
                xo8 = pa.tile([P, NCT * TCH], FP8, tag="xo8", name="xo8")
                for st in range(2):
                    nc.sync.dma_start(
                        xo8[:, st * 8 * TCH:(st + 1) * 8 * TCH].rearrange(
                            "p (a w) -> p a w", a=8),
                        xo8_ext.ap()[st * 8 * P:(st + 1) * 8 * P, :].rearrange(
                            "(a p) w -> p a w", p=P),
                    )
                xo8v = xo8[:].rearrange("p (a w) -> p a w", a=NCT)
                cq_own = pa.tile([P, NNL * TCH], FP8, tag="cqo", name="cqo")
                for ot in range(NNL):
                    acc = pwk.tile([P, TCH], F32, tag="acc", bufs=2,
                                   name="acc")
                    for cp in range(NCT // 2):
                        nc.tensor.matmul(
                            acc[:],
                            wdq8v[:, 2 * cp:2 * cp + 2, ot * P:(ot + 1) * P],
                            xo8v[:, 2 * cp:2 * cp + 2, :],
                            start=(cp == 0),
                            stop=(cp == NCT // 2 - 1),
                            perf_mode=DR,
                        )
                    nc.scalar.activation(
                        cq_own[:, ot * TCH:(ot + 1) * TCH], acc[:],
                        Copy, scale=QKS / (WUS * SX),
                    )
                nc.sync.dma_start(
                    agin_dram.ap().rearrange("(a p) w -> p a w", p=P),
                    cq_own[:].rearrange("p (a w) -> p a w", a=NNL),
                )
                nc.gpsimd.collective_compute(
                    "AllGather",
                    mybir.AluOpType.bypass,
                    replica_groups=[[0, 1, 2, 3], [4, 5, 6, 7]],
                    ins=[agin_dram.ap().opt()],
                    outs=[agout_dram.ap().opt()],
                )
                for nl in range(NNL):
                    for ch in range(NCH):
                        nc.sync.dma_start(
                            cq8[:, nl * T + ch * TCH:nl * T + (ch + 1) * TCH],
                            agout_dram.ap()[ch, nl * P:(nl + 1) * P, :],
                        )

                for ch in range(NCH):
                    tsl = slice(ch * TCH, (ch + 1) * TCH)
                    if ch + 2 < NCH:
                        dma_x(ch + 2)
                    xcv = xcs[ch][:].rearrange("p (a w) -> p a w", a=NCT)
                    x8v = x8s[ch][:].rearrange("p (a w) -> p a w", a=NCT)

                    # ckv (bf16): ct-outer over 4 concurrent PSUM groups
                    accs = [pbig.tile([P, 2 * TCH], F32, tag="big", name="big")
                            for _ in range(2)]
                    for ct in range(NCT):
                        for ot in range(NNL):
                            nc.tensor.matmul(
                                accs[ot // 2][:, (ot % 2) * TCH:(ot % 2 + 1) * TCH],
                                wdkv_sb[:, ct * NL + ot * P:ct * NL + (ot + 1) * P],
                                xcv[:, ct, :],
                                start=(ct == 0),
                                stop=(ct == NCT - 1),
                            )
                    with nc.allow_low_precision(reason="fp8 latents"):
                        for ot in range(NNL):
                            src = accs[ot // 2][:, (ot % 2) * TCH:
                                                (ot % 2 + 1) * TCH]
                            dst = slice(ot * T + ch * TCH,
                                        ot * T + (ch + 1) * TCH)
                            if ot % 2 == 0:
                                nc.vector.tensor_copy(ckv_sb[:, dst], src)
                                nc.scalar.activation(ckv8[:, dst], src, Copy,
                                                     scale=QKS)
                            else:
                                nc.scalar.copy(ckv_sb[:, dst], src)
                                nc.vector.tensor_scalar_mul(ckv8[:, dst], src,
                                                            QKS)

                    # kr (fp8 DoubleRow), rope on DVE
                    acck = pwk.tile([P, TCH], F32, tag="wo", bufs=1,
                                    name="wacc")
                    for cp in range(NCT // 2):
                        nc.tensor.matmul(
                            acck[0:RHD, :],
                            wkr8v[:, 2 * cp:2 * cp + 2, 0:RHD],
                            x8v[:, 2 * cp:2 * cp + 2, :],
                            start=(cp == 0),
                            stop=(cp == NCT // 2 - 1),
                            perf_mode=DR,
                        )
                    krst = pa.tile([RHD, TCH], BF16, tag="krst", bufs=2,
                                   name="krst")
                    nc.scalar.activation(krst[:], acck[0:RHD, :], Copy,
                                         scale=1.0 / (WUS * SX))
                    tmp = pa.tile([RHD, TCH], BF16, tag="rtmp", bufs=2,
                                  name="rtmp")
                    # kr is a single 64-row head: 32-row bands
                    nc.vector.tensor_mul(tmp[0:32, :], krst[32:64, :], sa[32:64, tsl])
                    nc.vector.tensor_mul(tmp[32:64, :], krst[32:64, :], ca[32:64, tsl])
                    nc.vector.tensor_mul(kr[0:32, tsl], krst[0:32, :], ca[0:32, tsl])
                    nc.vector.tensor_mul(kr[32:64, tsl], krst[0:32, :], sa[0:32, tsl])
                    nc.vector.tensor_sub(kr[0:32, tsl], kr[0:32, tsl], tmp[0:32, :])
                    nc.vector.tensor_add(kr[32:64, tsl], kr[32:64, tsl], tmp[32:64, :])

            # ------------- phase B/C: up-projections + attention -------------
            with (
                tc.tile_pool(name="ph", bufs=1) as ph,
                tc.tile_pool(name="pat", bufs=1) as pat,
            ):
                wuqv = wuq_sb[:].rearrange("p (a w) -> p a w", a=NNL)
                wukv = wuk_sb[:].rearrange("p (a w) -> p a w", a=NNL)
                wqrv = wqr_sb[:].rearrange("p (a w) -> p a w", a=NNL)
                cq8v = cq8[:].rearrange("p (a w) -> p a w", a=NNL)
                ckv8v = ckv8[:].rearrange("p (a w) -> p a w", a=NNL)

                # Q rope: packed 2 heads per matmul with host-planar layout
                # rows [0:64]=re(h0|h1), [64:128]=im(h0|h1).  The final
                # sub/add write the fp8 score operand directly (values are
                # pre-scaled by the qst copy).  Chunk ch is produced
                # just-in-time: ch0 up front, ch(tq+1) pipelined inside the
                # attention loop.
                def qr_chunk(ch):
                    Tsl = slice(T + ch * TCH, T + (ch + 1) * TCH)
                    for pr in range(HLOC // 2):
                        acc = pwk.tile([P, TCH], F32, tag="acc", bufs=2,
                                       name="acc")
                        for pr2 in range(NNL // 2):
                            nc.tensor.matmul(
                                acc[:],
                                wqrv[:, 2 * pr2:2 * pr2 + 2, pr * P:(pr + 1) * P],
                                cq8v[:, 2 * pr2:2 * pr2 + 2,
                                     ch * TCH:(ch + 1) * TCH],
                                start=(pr2 == 0),
                                stop=(pr2 == NNL // 2 - 1),
                                perf_mode=DR,
                            )
                        qst = pat.tile([P, TCH], BF16, tag="qst", bufs=2,
                                       name="qst")
                        # 1024*qr -> 16*qr so the rope output is fp8-ready
                        nc.scalar.activation(qst[:], acc[:], Copy,
                                             scale=1.0 / WUS)
                        tmp = pat.tile([P, TCH], BF16, tag="rtmp2", bufs=2,
                                       name="rtmp2")
                        # two-head 64-row multiplies, per-head 32-row fp8 writes
                        nc.vector.tensor_mul(tmp[0:64, :], qst[64:128, :],
                                             sa[64:128, ch * TCH:(ch + 1) * TCH])
                        nc.vector.tensor_mul(tmp[64:128, :], qst[64:128, :],
                                             ca[64:128, ch * TCH:(ch + 1) * TCH])
                        qre = pat.tile([P, TCH], BF16, tag="qre", bufs=2,
                                       name="qre")
                        nc.vector.tensor_mul(qre[0:64, :], qst[0:64, :],
                                             ca[0:64, ch * TCH:(ch + 1) * TCH])
                        nc.vector.tensor_mul(qre[64:128, :], qst[0:64, :],
                                             sa[0:64, ch * TCH:(ch + 1) * TCH])
                        with nc.allow_low_precision(reason="fp8 score operand"):
                            for sub in range(2):
                                h = pr * 2 + sub
                                ss = slice(sub * 32, sub * 32 + 32)
                                s2 = slice(64 + sub * 32, 64 + sub * 32 + 32)
                                nc.vector.tensor_sub(
                                    qk8[h][0:32, Tsl], qre[ss, :], tmp[ss, :])
                                nc.vector.tensor_add(
                                    qk8[h][32:64, Tsl], qre[s2, :], tmp[s2, :])

                qr_chunk(0)
                # interleave Q-content / V / K-content / K-rope so PE stays
                # fed and the converts spread across DVE / Pool / ACT.
                with nc.allow_low_precision(reason="fp8 score operand"):
                    for i in range(16):
                        h, ch = divmod(i, NCH)
                        tsl = slice(ch * TCH, (ch + 1) * TCH)
                        accq = pwk.tile([P, TCH], F32, tag="acc", bufs=2,
                                        name="acc")
                        for pr2 in range(NNL // 2):
                            nc.tensor.matmul(
                                accq[:],
                                wuqv[:, 2 * pr2:2 * pr2 + 2, h * P:(h + 1) * P],
                                cq8v[:, 2 * pr2:2 * pr2 + 2, tsl],
                                start=(pr2 == 0),
                                stop=(pr2 == NNL // 2 - 1),
                                perf_mode=DR,
                            )
                        nc.vector.tensor_scalar_mul(
                            qk8[h][:, tsl], accq[:], 1.0 / WUS)
                        # V (bf16) in natural [t, (head, hs)] layout
                        tb = i
                        accv = pwk.tile([P, TCH], F32, tag="acc", bufs=2,
                                        name="acc")
                        for nl in range(NNL):
                            nc.tensor.matmul(
                                accv[:],
                                ckv_sb[:, nl * T + tb * P:nl * T + (tb + 1) * P],
                                wuv_sb[:, nl * HLOC * HS:(nl + 1) * HLOC * HS],
                                start=(nl == 0),
                                stop=(nl == NNL - 1),
                            )
                        if i % 2 == 0:
                            nc.vector.tensor_copy(
                                vv[:, tb * TCH:(tb + 1) * TCH], accv[:])
                        else:
                            nc.scalar.copy(
                                vv[:, tb * TCH:(tb + 1) * TCH], accv[:])
                        acck = pwk.tile([P, TCH], F32,
                                        tag=("wo" if i % 2 else "dn"), bufs=1,
                                        name="wacc")
                        for pr2 in range(NNL // 2):
                            nc.tensor.matmul(
                                acck[:],
                                wukv[:, 2 * pr2:2 * pr2 + 2, h * P:(h + 1) * P],
                                ckv8v[:, 2 * pr2:2 * pr2 + 2, tsl],
                                start=(pr2 == 0),
                                stop=(pr2 == NNL // 2 - 1),
                                perf_mode=DR,
                            )
                        nc.scalar.activation(
                            kk8[h][:, tsl], acck[:], Copy, scale=1.0 / WUS)
                        rsl = slice(T + ch * TCH, T + (ch + 1) * TCH)
                        nc.gpsimd.tensor_scalar_mul(
                            kk8[h][0:RHD, rsl], kr[:, tsl], QKS)

                dnorm = pwk.tile([P, TCH], F32, tag="dn", bufs=1, name="dn")

                def emit_wo_tile(tq_prev, cs, ohs, cp_eng=0, tag="wo"):
                    qsl2 = slice(tq_prev * TCH, (tq_prev + 1) * TCH)
                    acc = pwk.tile([P, TCH], F32, tag=tag,
                                   bufs=(1 if tag == "wo" else 2), name="wacc")
                    for hh in range(HLOC):
                        nc.tensor.matmul(
                            acc[:],
                            wo_sb[:, hh * C + cs * P:hh * C + (cs + 1) * P],
                            ohs[hh][:],
                            start=(hh == 0),
                            stop=(hh == HLOC - 1),
                        )
                    ot = pat.tile([P, TCH], BF16, tag="ot", bufs=3, name="ot")
                    if cp_eng % 2 == 0:
                        nc.vector.tensor_copy(ot[:], acc[:])
                    else:
                        nc.scalar.copy(ot[:], acc[:])
                    nc.sync.dma_start(
                        out_ext.ap()[cs * P:(cs + 1) * P, qsl2],
                        ot[:],
                    )

                oh_prev = None
                pending_norm = [None]
                ghead = [0]

                # bf16 view of the dnorm bank: den_row [1, 512] lives at
                # bf16 cols [16:528] (den cols occupy f32 cols [0:8]).
                dnorm_bf = dnorm[:].bitcast(BF16)

                def make_norm(outU_, oh_t, base, npair_):
                    def run():
                        if DEN_TRICK:
                            den_sb = pat.tile([P, 4], BF16, tag="dsb", bufs=2,
                                              name="dsb")
                            nc.vector.tensor_copy(den_sb[:],
                                                  dnorm[:, base:base + 4])
                            for pr_ in range(1, npair_):
                                nc.vector.tensor_add(
                                    den_sb[:], den_sb[:],
                                    dnorm[:, base + 4 * pr_:
                                          base + 4 * pr_ + 4])
                            for qb in range(4):
                                nc.tensor.matmul(
                                    dnorm_bf[0:1,
                                             128 + qb * P:128 + (qb + 1) * P],
                                    den_sb[:, qb:qb + 1], ident[:],
                                    is_transpose=True,
                                    skip_group_check=True)
                            rsrc = dnorm_bf[0:1, 128:128 + TCH]
                        else:
                            rsrc = dnorm[0:1, 0:TCH]
                        recipb = pat.tile([1, TCH], BF16, tag="rcb",
                                          name="rcb", bufs=2)
                        with nc.allow_low_precision(reason="recip fits bf16"):
                            nc.vector.reciprocal(recipb[:], rsrc)
                        bcast = pat.tile([P, TCH], BF16, tag="bcs", bufs=2,
                                         name="bcs")
                        nc.gpsimd.partition_broadcast(bcast[:], recipb[:])
                        nc.vector.tensor_mul(oh_t[:], outU_[:], bcast[:])
                    return run

                for tq in range(NCH):
                    oh_cur = [pat.tile([P, TCH], BF16, tag=f"oh{i}",
                                       name=f"oh{i}", bufs=2)
                              for i in range(HLOC)]
                    pairs_total = HLOC * 2 * (tq + 1)
                    pairs_done = 0
                    wo_state = [0]

                    def wo_pace():
                        if tq == 0:
                            return
                        target = min(16, (pairs_done * 16 + pairs_total - 1)
                                     // pairs_total + 1)
                        while wo_state[0] < target:
                            emit_wo_tile(tq - 1, wo_state[0], oh_prev,
                                         cp_eng=0)
                            wo_state[0] += 1

                    for h in range(HLOC):
                        outU = pwk.tile([P, TCH], F32, tag="acc", bufs=2,
                                        name="acc")
                        kkv = kk8[h][:].rearrange("p (a t) -> p a t", a=2)
                        qkv = qk8[h][:].rearrange("p (a t) -> p a t", a=2)
                        npair = 2 * (tq + 1)
                        base = (ghead[0] % 2) * 32
                        ghead[0] += 1

                        # den accumulation chains: dnorm[:, base+qb] over
                        # k-tiles, probability tile as stationary, ones col as
                        # moving (1-row matmuls).  Every qb chain starts at
                        # kt=0 and ends at its diagonal tile kt = 4*tq + qb.
                        def emit_dp(p):
                            Ptp, kts, pr_ = p
                            for i, kt in enumerate(kts):
                                diag = kt // 4 == tq
                                ks = kt % 4
                                off = ks * P if diag else 0
                                nc.tensor.matmul(
                                    outU[:, off:],
                                    vv[:, kt * TCH + h * P:
                                       kt * TCH + (h + 1) * P],
                                    Ptp[:, i * TCH + off:(i + 1) * TCH],
                                    start=(kt == 0),
                                    stop=(kt == 4 * tq + 3),
                                    skip_group_check=True,
                                )
                            if not DEN_TRICK:
                                for i, kt in enumerate(kts):
                                    diag = kt // 4 == tq
                                    off = (kt % 4) * P if diag else 0
                                    nc.tensor.matmul(
                                        dnorm[0:1, off:TCH],
                                        ones[:, 0:1],
                                        Ptp[:, i * TCH + off:(i + 1) * TCH],
                                        start=(kt == 0),
                                        stop=(kt == 4 * tq + 3),
                                        skip_group_check=True,
                                    )
                                return
                            # per-(pair, qb) committed chains: at most one
                            # open accumulation chain per PSUM bank at any
                            # time (a start=True matmul wipes the bank's
                            # uncommitted accumulator state).
                            for qb in range(4):
                                col = base + pr_ * 4 + qb
                                valid = [(i, kt) for i, kt in enumerate(kts)
                                         if not (kt // 4 == tq
                                                 and qb < kt % 4)]
                                if not valid:
                                    nc.tensor.matmul(
                                        dnorm[:, col:col + 1],
                                        zers[:], ones[:, 0:1],
                                        start=True, stop=True,
                                        skip_group_check=True,
                                    )
                                    continue
                                for j, (i, kt) in enumerate(valid):
                                    nc.tensor.matmul(
                                        dnorm[:, col:col + 1],
                                        Ptp[:, i * TCH + qb * P:
                                            i * TCH + (qb + 1) * P],
                                        ones[:, 0:1],
                                        start=(j == 0),
                                        stop=(j == len(valid) - 1),
                                        skip_group_check=True,
                                    )

                        prev = None
                        for pr in range(npair):
                            kt0 = 2 * pr
                            kt1 = 2 * pr + 1
                            diag0 = kt0 // 4 == tq
                            diag1 = kt1 // 4 == tq
                            off0 = (kt0 % 4) * P if diag0 else 0
                            off1 = (kt1 % 4) * P if diag1 else 0
                            ST2 = pbig.tile([P, 2 * TCH], F32, tag="big",
                                            name="big")
                            nc.tensor.matmul(
                                ST2[:, off0:TCH],
                                kkv[:, :, kt0 * P:(kt0 + 1) * P],
                                qkv[:, :, tq * TCH + off0:(tq + 1) * TCH],
                                start=True, stop=True,
                                perf_mode=DR,
                            )
                            nc.tensor.matmul(
                                ST2[:, TCH + off1:2 * TCH],
                                kkv[:, :, kt1 * P:(kt1 + 1) * P],
                                qkv[:, :, tq * TCH + off1:(tq + 1) * TCH],
                                start=True, stop=True,
                                perf_mode=DR,
                            )
                            Pt = pat.tile([P, 2 * TCH], BF16, tag="pt",
                                          bufs=5, name="pt")
                            if diag0:
                                nc.scalar.activation(Pt[:, off0:TCH],
                                                     ST2[:, off0:TCH],
                                                     Exp, scale=SCALE8)
                                nc.scalar.activation(Pt[:, TCH + off1:],
                                                     ST2[:, TCH + off1:],
                                                     Exp, scale=SCALE8)
                                # causal boundary: multiplicative 0/1 mask on
                                # the diagonal 128-block, on the Pool engine
                                nc.gpsimd.tensor_mul(
                                    Pt[:, off0:off0 + P],
                                    Pt[:, off0:off0 + P], m2b[:])
                                nc.gpsimd.tensor_mul(
                                    Pt[:, TCH + off1:TCH + off1 + P],
                                    Pt[:, TCH + off1:TCH + off1 + P], m2b[:])
                            else:
                                nc.scalar.activation(Pt[:], ST2[:],
                                                     Exp, scale=SCALE8)
                            if pr == 1 and pending_norm[0] is not None:
                                pending_norm[0]()
                                pending_norm[0] = None
                            if prev is not None:
                                emit_dp(prev)
                                pairs_done += 1
                                wo_pace()
                            prev = (Pt, (kt0, kt1), pr)
                        emit_dp(prev)
                        pairs_done += 1
                        wo_pace()
                        pending_norm[0] = make_norm(outU, oh_cur[h], base, npair)
                    if tq + 1 < NCH:
                        qr_chunk(tq + 1)
                    oh_prev = oh_cur
                # flush the last head's normalization, then the final chunk's
                # W_o with copies alternating engines to drain fast
                if pending_norm[0] is not None:
                    pending_norm[0]()
                    pending_norm[0] = None
                for cs in range(C // P):
                    emit_wo_tile(NCH - 1, cs, oh_prev, cp_eng=cs % 2,
                                 tag=("wo" if cs % 2 else "acc"))

    nc.compile()
    return nc


def _get_nc():
    if "nc" not in _NC_CACHE:
        _NC_CACHE["nc"] = build()
    return _NC_CACHE["nc"]


def kernel(x, freqs_cos, freqs_sin, W_dq, W_uq, W_dkv, W_uk, W_uv, W_qr, W_kr,
           W_o, trace=False, **trace_kwargs):
    nc = _get_nc()
    bf = ml_dtypes.bfloat16
    f8 = ml_dtypes.float8_e4m3fn
    cT8 = lambda a: np.ascontiguousarray(
        (np.asarray(a, dtype=np.float32).T * WUS).astype(f8))
    f32 = lambda a: np.asarray(a, dtype=np.float32)
    cT = lambda a: np.ascontiguousarray(f32(a).T.astype(bf))

    x = f32(x)
    cos = f32(freqs_cos)
    sin = f32(freqs_sin)

    # host-side preprocessing (shared across cores)
    wdq8 = cT8(W_dq)                      # [C, NL] fp8
    wdkvT = cT(W_dkv)                     # [C, NL] bf16
    perm_r = np.concatenate([np.arange(0, RHD, 2), np.arange(1, RHD, 2)])
    wkr8 = cT8(f32(W_kr)[perm_r])         # [C, RHD] fp8 planar
    # rope tables: 4x-planar duplicated [128, T]
    caT = np.ascontiguousarray(
        np.tile(cos.T, (4, 1)).astype(bf))  # [128, T]
    saT = np.ascontiguousarray(
        np.tile(sin.T, (4, 1)).astype(bf))
    xTb = [np.ascontiguousarray(x[b].T.astype(bf)) for b in range(B)]
    x8b = [np.ascontiguousarray((x[b].T * SX).astype(f8)) for b in range(B)]

    W_qr_f = f32(W_qr)
    in_maps = []
    for c in range(8):
        b, r = divmod(c, 4)
        hsl = slice(r * HLOC * HS, (r + 1) * HLOC * HS)
        # W_qr rows per head pair: [h0 re(32) | h1 re(32) | h0 im(32) | h1 im(32)]
        wqr_rows = []
        for pr in range(HLOC // 2):
            h0 = r * HLOC * RHD + (2 * pr) * RHD
            h1 = r * HLOC * RHD + (2 * pr + 1) * RHD
            re0 = W_qr_f[h0:h0 + RHD][np.arange(0, RHD, 2)]
            re1 = W_qr_f[h1:h1 + RHD][np.arange(0, RHD, 2)]
            im0 = W_qr_f[h0:h0 + RHD][np.arange(1, RHD, 2)]
            im1 = W_qr_f[h1:h1 + RHD][np.arange(1, RHD, 2)]
            wqr_rows += [re0, re1, im0, im1]
        wqrT = np.ascontiguousarray(
            (np.concatenate(wqr_rows, axis=0).T * WUS).astype(f8))  # [NL, 256]
        in_maps.append({
            "xT": xTb[b],
            "x8": x8b[b],
            "xo8": np.ascontiguousarray(x8b[b][:, r * TCH:(r + 1) * TCH]),
            "wdq8": wdq8, "wdkvT": wdkvT, "wkr8": wkr8,
            "wuqT": cT8(f32(W_uq)[hsl]),
            "wukT": cT8(f32(W_uk)[hsl]),
            "wuvT": cT(f32(W_uv)[hsl]),
            "wqrT": wqrT,
            "woT": cT(f32(W_o)[:, hsl]),
            "caT": caT, "saT": saT,
        })
    res = run_bass_kernel_spmd(nc, in_maps, core_ids=list(range(8)),
                               trace=trace, **trace_kwargs)
    out = np.zeros((B, T, C), dtype=np.float32)
    for c in range(8):
        b = c // 4
        out[b] += res.results[c]["out"].astype(np.float32).T
    kernel.last_result = res
    return out


# revision 25
# speedup vs baseline: 1.0156x; 1.0156x over previous
"""MLA-style attention kernel for 8 TRN2 NeuronCores (v5).

Sharding: core c -> batch b = c//4, heads r*4..r*4+3 where r = c%4.

v5 vs v3-baseline: the AllGather is gone -- every core computes cq for
the FULL T with fp8 DoubleRow matmuls (same PE cost as the old own-chunk
bf16 cq, no collective, no dependency stall).  The score path is fully
fp8-DR (cq, kr, K up-projection contract in fp8 pairs); the value path
(ckv -> V -> PV -> W_o) stays bf16 end-to-end for accuracy.  The softmax
denominator is computed with 1-row matmuls (probability tile stationary,
a ones column moving), transposed to row form with one PE transpose, so
the old 512-row ones-matmuls disappear.  exp runs on [128,1024] score
pairs (two k-tiles per 2-bank PSUM tile) to halve ACT instruction count.
W_o tiles for the previous chunk are interleaved into the attention pair
loop so PE never waits on exp; output copies run on the Pool engine.
Rope multiplies run on 64-row two-head bands and the final sub/add write
fp8 score operands directly (no separate convert).  Q/V/K up-projection
chains are interleaved round-robin with their converts spread across
DVE/Pool/ACT so no single engine paces phase B.
"""
import math
import numpy as np
import ml_dtypes

import concourse.bass as bass
import concourse.bacc as bacc
import concourse.mybir as mybir
import concourse.tile as tile
from concourse.bass_utils import run_bass_kernel_spmd

F32 = mybir.dt.float32
BF16 = mybir.dt.bfloat16
FP8 = mybir.dt.float8e4
QKS = 16.0            # fp8 scale for q/k operands
WUS = 64.0            # fp8 scale for weights
SX = 4.0              # fp8 scale for x (score path)
Exp = mybir.ActivationFunctionType.Exp
Copy = mybir.ActivationFunctionType.Copy

B, T, C = 2, 2048, 2048
H = 16
HS = 128
NL = 512
RHD = 64
HLOC = 4              # heads per core
P = 128
NNL = NL // P         # 4 latent p-tiles
TCH = 512
NCH = T // TCH        # 4 T-chunks
NCT = C // P          # 16 contraction p-tiles over C
SCALE = 1.0 / math.sqrt(HS + RHD)
SCALE8 = SCALE / (QKS * QKS)
NEG = -1.0e30
DR = mybir.MatmulPerfMode.DoubleRow

_NC_CACHE = {}
DEN_TRICK = True


def build():
    nc = bacc.Bacc("TRN2", target_bir_lowering=False, debug=False, num_devices=8)

    xT_ext = nc.dram_tensor("xT", [C, T], BF16, kind="ExternalInput")
    x8_ext = nc.dram_tensor("x8", [C, T], FP8, kind="ExternalInput")
    wdq8_ext = nc.dram_tensor("wdq8", [C, NL], FP8, kind="ExternalInput")
    wdkvT_ext = nc.dram_tensor("wdkvT", [C, NL], BF16, kind="ExternalInput")
    wkr8_ext = nc.dram_tensor("wkr8", [C, RHD], FP8, kind="ExternalInput")
    wuqT_ext = nc.dram_tensor("wuqT", [NL, HLOC * HS], FP8, kind="ExternalInput")
    wukT_ext = nc.dram_tensor("wukT", [NL, HLOC * HS], FP8, kind="ExternalInput")
    wuvT_ext = nc.dram_tensor("wuvT", [NL, HLOC * HS], BF16, kind="ExternalInput")
    wqrT_ext = nc.dram_tensor("wqrT", [NL, HLOC * RHD], FP8, kind="ExternalInput")
    woT_ext = nc.dram_tensor("woT", [HLOC * HS, C], BF16, kind="ExternalInput")
    caT_ext = nc.dram_tensor("caT", [P, T], BF16, kind="ExternalInput")
    saT_ext = nc.dram_tensor("saT", [P, T], BF16, kind="ExternalInput")
    out_ext = nc.dram_tensor("out", [C, T], BF16, kind="ExternalOutput")
    xo8_ext = nc.dram_tensor("xo8", [C, TCH], FP8, kind="ExternalInput")
    agin_dram = nc.dram_tensor("agin", [NL, TCH], FP8)
    agout_dram = nc.dram_tensor("agout", [NCH, NL, TCH], FP8)

    ones_dram = nc.inline_tensor(np.ones((P, P), dtype=ml_dtypes.bfloat16),
                                 name="onesc")
    ident_dram = nc.inline_tensor(np.eye(P, dtype=ml_dtypes.bfloat16),
                                  name="identc")
    # boundary mask for the diagonal 128-col sub-block of S^T tiles [k, q]:
    # m2[jj, u] = 0 if u >= jj else -1e30
    m2 = np.zeros((P, P), dtype=ml_dtypes.bfloat16)
    for jj in range(P):
        m2[jj, jj:] = 1.0
    m2_dram = nc.inline_tensor(m2, name="m2c")

    with tile.TileContext(nc) as tc:
        with (
            tc.tile_pool(name="pers", bufs=1) as pers,
            tc.tile_pool(name="pbig", bufs=2, space="PSUM") as pbig,
            tc.tile_pool(name="pwk", bufs=1, space="PSUM") as pwk,
        ):
            ones = pers.tile([P, P], BF16, tag="ones", name="ones")
            zers = pers.tile([P, P], BF16, tag="zers", name="zers")
            nc.gpsimd.memset(zers[:], 0.0)
            ident = pers.tile([P, P], BF16, tag="ident", name="ident")
            m2b = pers.tile([P, P], BF16, tag="m2b", name="m2b")

            ca = pers.tile([P, T], BF16, tag="ca", name="ca")
            sa = pers.tile([P, T], BF16, tag="sa", name="sa")
            # PE p-state warmup: dummy matmuls on not-yet-loaded tiles keep the
            # tensor engine continuously busy through the initial DMA wait so
            # real matmuls start at full clock.
            for wi in range(14):
                warm = pbig.tile([P, 2 * TCH], F32, tag="big", name="big")
                nc.tensor.matmul(
                    warm[:, 0:TCH], ca[:, 0:P], sa[:, 0:TCH],
                    start=True, stop=True, skip_group_check=True,
                )

            # persistent activations
            ckv_sb = pers.tile([P, NNL * T], BF16, tag="ckv", name="ckv")
            ckv8 = pers.tile([P, NNL * T], FP8, tag="ckv8", name="ckv8")
            cq8 = pers.tile([P, NNL * T], FP8, tag="cq8", name="cq8")
            kr = pers.tile([RHD, T], BF16, tag="kr", name="kr")

            # score operands + V live in the pers pool so their memsets can
            # run at t=0 under the initial DMA window.
            qk8 = [pers.tile([P, 2 * T], FP8, tag=f"qk8{i}", name=f"qk8{i}")
                   for i in range(HLOC)]
            kk8 = [pers.tile([P, 2 * T], FP8, tag=f"kk8{i}", name=f"kk8{i}")
                   for i in range(HLOC)]
            for i in range(HLOC):
                nc.gpsimd.memset(qk8[i][RHD:P, T:2 * T], 0.0)
                nc.gpsimd.memset(kk8[i][RHD:P, T:2 * T], 0.0)
            vv = pers.tile([P, (T // P) * TCH], BF16, tag="vv", name="vv")

            # up/out-projection weights (preloaded early, used later)
            wuq_sb = pers.tile([P, NNL * HLOC * HS], FP8, tag="wuq", name="wuq")
            wuk_sb = pers.tile([P, NNL * HLOC * HS], FP8, tag="wuk", name="wuk")
            wuv_sb = pers.tile([P, NNL * HLOC * HS], BF16, tag="wuv", name="wuv")
            wqr_sb = pers.tile([P, NNL * HLOC * RHD], FP8, tag="wqr", name="wqr")
            wo_sb = pers.tile([P, HLOC * C], BF16, tag="wo", name="wo")

            # ---------------- phase A: down-projections ----------------
            with tc.tile_pool(name="pa", bufs=1) as pa:
                wdq8_sb = pa.tile([P, NCT * NL], FP8, tag="wdq", name="wdq")
                wdkv_sb = pa.tile([P, NCT * NL], BF16, tag="wdkv", name="wdkv")
                wkr8_sb = pa.tile([P, NCT * RHD], FP8, tag="wkr", name="wkr")
                xcs = [pa.tile([P, NCT * TCH], BF16, tag="xc", bufs=2,
                               name="xc") for _ in range(NCH)]
                x8s = [pa.tile([P, NCT * TCH], FP8, tag="x8c", bufs=2,
                               name="x8c") for _ in range(NCH)]

                def dma_x(ch):
                    tsl = slice(ch * TCH, (ch + 1) * TCH)
                    for st in range(4):
                        nc.sync.dma_start(
                            xcs[ch][:, st * 4 * TCH:(st + 1) * 4 * TCH].rearrange(
                                "p (a w) -> p a w", a=4),
                            xT_ext.ap()[st * 4 * P:(st + 1) * 4 * P, tsl].rearrange(
                                "(a p) w -> p a w", p=P),
                        )
                    for st in range(2):
                        nc.sync.dma_start(
                            x8s[ch][:, st * 8 * TCH:(st + 1) * 8 * TCH].rearrange(
                                "p (a w) -> p a w", a=8),
                            x8_ext.ap()[st * 8 * P:(st + 1) * 8 * P, tsl].rearrange(
                                "(a p) w -> p a w", p=P),
                        )

                # critical-path first: wdkv + chunk-0 x stripes interleaved
                for st in range(4):
                    nc.sync.dma_start(
                        wdkv_sb[:, st * 4 * NL:(st + 1) * 4 * NL].rearrange(
                            "p (a w) -> p a w", a=4),
                        wdkvT_ext.ap()[st * 4 * P:(st + 1) * 4 * P, :].rearrange(
                            "(a p) w -> p a w", p=P),
                    )
                    nc.sync.dma_start(
                        xcs[0][:, st * 4 * TCH:(st + 1) * 4 * TCH].rearrange(
                            "p (a w) -> p a w", a=4),
                        xT_ext.ap()[st * 4 * P:(st + 1) * 4 * P, 0:TCH].rearrange(
                            "(a p) w -> p a w", p=P),
                    )
                for st in range(2):
                    nc.sync.dma_start(
                        wdq8_sb[:, st * 8 * NL:(st + 1) * 8 * NL].rearrange(
                            "p (a w) -> p a w", a=8),
                        wdq8_ext.ap()[st * 8 * P:(st + 1) * 8 * P, :].rearrange(
                            "(a p) w -> p a w", p=P),
                    )
                    nc.sync.dma_start(
                        x8s[0][:, st * 8 * TCH:(st + 1) * 8 * TCH].rearrange(
                            "p (a w) -> p a w", a=8),
                        x8_ext.ap()[st * 8 * P:(st + 1) * 8 * P, 0:TCH].rearrange(
                            "(a p) w -> p a w", p=P),
                    )
                nc.sync.dma_start(
                    wkr8_sb[:].rearrange("p (a w) -> p a w", a=NCT),
                    wkr8_ext.ap().rearrange("(a p) w -> p a w", p=P),
                )
                dma_x(1)
                # non-critical loads on the Pool (SWDGE) queue
                nc.gpsimd.dma_start(out=ones[:], in_=ones_dram.ap())
                nc.gpsimd.dma_start(out=ident[:], in_=ident_dram.ap())
                nc.gpsimd.dma_start(out=m2b[:], in_=m2_dram.ap())
                nc.gpsimd.dma_start(out=ca[:], in_=caT_ext.ap())
                nc.gpsimd.dma_start(out=sa[:], in_=saT_ext.ap())
                # phase-B weights: needed only after ~70us, issue behind x
                for ext, sb in ((wuqT_ext, wuq_sb), (wqrT_ext, wqr_sb),
                                (wukT_ext, wuk_sb), (wuvT_ext, wuv_sb)):
                    nc.sync.dma_start(
                        sb[:].rearrange("p (a w) -> p a w", a=NNL),
                        ext.ap().rearrange("(a p) w -> p a w", p=P),
                    )
                nc.sync.dma_start(
                    wo_sb[:].rearrange("p (a w) -> p a w", a=HLOC),
                    woT_ext.ap().rearrange("(a p) w -> p a w", p=P),
                )

                wdq8v = wdq8_sb[:].rearrange("p (a w) -> p a w", a=NCT)
                wkr8v = wkr8_sb[:].rearrange("p (a w) -> p a w", a=NCT)

                # cq for the OWN T-chunk only (host stages xo8 per core);
                # AllGathered in fp8 across the 4-core group and consumed
                # directly by the fp8 Q up-projections.  The collective runs
                # under the whole of phase A.
                xo8 = pa.tile([P, NCT * TCH], FP8, tag="xo8", name="xo8")
                for st in range(2):
                    nc.sync.dma_start(
                        xo8[:, st * 8 * TCH:(st + 1) * 8 * TCH].rearrange(
                            "p (a w) -> p a w", a=8),
                        xo8_ext.ap()[st * 8 * P:(st + 1) * 8 * P, :].rearrange(
                            "(a p) w -> p a w", p=P),
                    )
                xo8v = xo8[:].rearrange("p (a w) -> p a w", a=NCT)
                cq_own = pa.tile([P, NNL * TCH], FP8, tag="cqo", name="cqo")
                for ot in range(NNL):
                    acc = pwk.tile([P, TCH], F32, tag="acc", bufs=2,
                                   name="acc")
                    for cp in range(NCT // 2):
                        nc.tensor.matmul(
                            acc[:],
                            wdq8v[:, 2 * cp:2 * cp + 2, ot * P:(ot + 1) * P],
                            xo8v[:, 2 * cp:2 * cp + 2, :],
                            start=(cp == 0),
                            stop=(cp == NCT // 2 - 1),
                            perf_mode=DR,
                        )
                    nc.scalar.activation(
                        cq_own[:, ot * TCH:(ot + 1) * TCH], acc[:],
                        Copy, scale=QKS / (WUS * SX),
                    )
                nc.sync.dma_start(
                    agin_dram.ap().rearrange("(a p) w -> p a w", p=P),
                    cq_own[:].rearrange("p (a w) -> p a w", a=NNL),
                )
                nc.gpsimd.collective_compute(
                    "AllGather",
                    mybir.AluOpType.bypass,
                    replica_groups=[[0, 1, 2, 3], [4, 5, 6, 7]],
                    ins=[agin_dram.ap().opt()],
                    outs=[agout_dram.ap().opt()],
                )
                for ch in range(NCH):
                    tsl = slice(ch * TCH, (ch + 1) * TCH)
                    if ch + 2 < NCH:
                        dma_x(ch + 2)
                    xcv = xcs[ch][:].rearrange("p (a w) -> p a w", a=NCT)
                    x8v = x8s[ch][:].rearrange("p (a w) -> p a w", a=NCT)

                    # ckv (bf16): ct-outer over 4 concurrent PSUM groups
                    accs = [pbig.tile([P, 2 * TCH], F32, tag="big", name="big")
                            for _ in range(2)]
                    for ct in range(NCT):
                        for ot in range(NNL):
                            nc.tensor.matmul(
                                accs[ot // 2][:, (ot % 2) * TCH:(ot % 2 + 1) * TCH],
                                wdkv_sb[:, ct * NL + ot * P:ct * NL + (ot + 1) * P],
                                xcv[:, ct, :],
                                start=(ct == 0),
                                stop=(ct == NCT - 1),
                            )
                    with nc.allow_low_precision(reason="fp8 latents"):
                        for ot in range(NNL):
                            src = accs[ot // 2][:, (ot % 2) * TCH:
                                                (ot % 2 + 1) * TCH]
                            dst = slice(ot * T + ch * TCH,
                                        ot * T + (ch + 1) * TCH)
                            if ot % 2 == 0:
                                nc.vector.tensor_copy(ckv_sb[:, dst], src)
                                nc.scalar.activation(ckv8[:, dst], src, Copy,
                                                     scale=QKS)
                            else:
                                nc.scalar.copy(ckv_sb[:, dst], src)
                                nc.vector.tensor_scalar_mul(ckv8[:, dst], src,
                                                            QKS)

                    # kr (fp8 DoubleRow), rope on DVE
                    acck = pwk.tile([P, TCH], F32, tag="wo", bufs=1,
                                    name="wacc")
                    for cp in range(NCT // 2):
                        nc.tensor.matmul(
                            acck[0:RHD, :],
                            wkr8v[:, 2 * cp:2 * cp + 2, 0:RHD],
                            x8v[:, 2 * cp:2 * cp + 2, :],
                            start=(cp == 0),
                            stop=(cp == NCT // 2 - 1),
                            perf_mode=DR,
                        )
                    krst = pa.tile([RHD, TCH], BF16, tag="krst", bufs=2,
                                   name="krst")
                    nc.scalar.activation(krst[:], acck[0:RHD, :], Copy,
                                         scale=1.0 / (WUS * SX))
                    tmp = pa.tile([RHD, TCH], BF16, tag="rtmp", bufs=2,
                                  name="rtmp")
                    # kr is a single 64-row head: 32-row bands
                    nc.vector.tensor_mul(tmp[0:32, :], krst[32:64, :], sa[32:64, tsl])
                    nc.vector.tensor_mul(tmp[32:64, :], krst[32:64, :], ca[32:64, tsl])
                    nc.vector.tensor_mul(kr[0:32, tsl], krst[0:32, :], ca[0:32, tsl])
                    nc.vector.tensor_mul(kr[32:64, tsl], krst[0:32, :], sa[0:32, tsl])
                    nc.vector.tensor_sub(kr[0:32, tsl], kr[0:32, tsl], tmp[0:32, :])
                    nc.vector.tensor_add(kr[32:64, tsl], kr[32:64, tsl], tmp[32:64, :])

                # gathered cq8 lands mid-phase-A; one rearranged DMA placed
                # after the chunk loop so the SP queue never stalls on the
                # collective
                for nl in range(NNL):
                    nc.sync.dma_start(
                        cq8[:, nl * T:(nl + 1) * T].rearrange(
                            "p (ch w) -> p ch w", ch=NCH),
                        agout_dram.ap()[:, nl * P:(nl + 1) * P, :].rearrange(
                            "ch p w -> p ch w"),
                    )

            # ------------- phase B/C: up-projections + attention -------------
            with (
                tc.tile_pool(name="ph", bufs=1) as ph,
                tc.tile_pool(name="pat", bufs=1) as pat,
            ):
                wuqv = wuq_sb[:].rearrange("p (a w) -> p a w", a=NNL)
                wukv = wuk_sb[:].rearrange("p (a w) -> p a w", a=NNL)
                wqrv = wqr_sb[:].rearrange("p (a w) -> p a w", a=NNL)
                cq8v = cq8[:].rearrange("p (a w) -> p a w", a=NNL)
                ckv8v = ckv8[:].rearrange("p (a w) -> p a w", a=NNL)

                # Q rope: packed 2 heads per matmul with host-planar layout
                # rows [0:64]=re(h0|h1), [64:128]=im(h0|h1).  The final
                # sub/add write the fp8 score operand directly (values are
                # pre-scaled by the qst copy).  Chunk ch is produced
                # just-in-time: ch0 up front, ch(tq+1) pipelined inside the
                # attention loop.
                def qr_chunk(ch):
                    Tsl = slice(T + ch * TCH, T + (ch + 1) * TCH)
                    for pr in range(HLOC // 2):
                        acc = pwk.tile([P, TCH], F32, tag="acc", bufs=2,
                                       name="acc")
                        for pr2 in range(NNL // 2):
                            nc.tensor.matmul(
                                acc[:],
                                wqrv[:, 2 * pr2:2 * pr2 + 2, pr * P:(pr + 1) * P],
                                cq8v[:, 2 * pr2:2 * pr2 + 2,
                                     ch * TCH:(ch + 1) * TCH],
                                start=(pr2 == 0),
                                stop=(pr2 == NNL // 2 - 1),
                                perf_mode=DR,
                            )
                        qst = pat.tile([P, TCH], BF16, tag="qst", bufs=2,
                                       name="qst")
                        # 1024*qr -> 16*qr so the rope output is fp8-ready
                        nc.scalar.activation(qst[:], acc[:], Copy,
                                             scale=1.0 / WUS)
                        tmp = pat.tile([P, TCH], BF16, tag="rtmp2", bufs=2,
                                       name="rtmp2")
                        # two-head 64-row multiplies, per-head 32-row fp8 writes
                        nc.vector.tensor_mul(tmp[0:64, :], qst[64:128, :],
                                             sa[64:128, ch * TCH:(ch + 1) * TCH])
                        nc.vector.tensor_mul(tmp[64:128, :], qst[64:128, :],
                                             ca[64:128, ch * TCH:(ch + 1) * TCH])
                        qre = pat.tile([P, TCH], BF16, tag="qre", bufs=2,
                                       name="qre")
                        nc.vector.tensor_mul(qre[0:64, :], qst[0:64, :],
                                             ca[0:64, ch * TCH:(ch + 1) * TCH])
                        nc.vector.tensor_mul(qre[64:128, :], qst[0:64, :],
                                             sa[0:64, ch * TCH:(ch + 1) * TCH])
                        with nc.allow_low_precision(reason="fp8 score operand"):
                            for sub in range(2):
                                h = pr * 2 + sub
                                ss = slice(sub * 32, sub * 32 + 32)
                                s2 = slice(64 + sub * 32, 64 + sub * 32 + 32)
                                nc.vector.tensor_sub(
                                    qk8[h][0:32, Tsl], qre[ss, :], tmp[ss, :])
                                nc.vector.tensor_add(
                                    qk8[h][32:64, Tsl], qre[s2, :], tmp[s2, :])

                qr_chunk(0)
                # interleave Q-content / V / K-content / K-rope so PE stays
                # fed and the converts spread across DVE / Pool / ACT.
                with nc.allow_low_precision(reason="fp8 score operand"):
                    for i in range(16):
                        h, ch = divmod(i, NCH)
                        tsl = slice(ch * TCH, (ch + 1) * TCH)
                        accq = pwk.tile([P, TCH], F32, tag="acc", bufs=2,
                                        name="acc")
                        for pr2 in range(NNL // 2):
                            nc.tensor.matmul(
                                accq[:],
                                wuqv[:, 2 * pr2:2 * pr2 + 2, h * P:(h + 1) * P],
                                cq8v[:, 2 * pr2:2 * pr2 + 2, tsl],
                                start=(pr2 == 0),
                                stop=(pr2 == NNL // 2 - 1),
                                perf_mode=DR,
                            )
                        nc.vector.tensor_scalar_mul(
                            qk8[h][:, tsl], accq[:], 1.0 / WUS)
                        # V (bf16) in natural [t, (head, hs)] layout
                        tb = i
                        accv = pwk.tile([P, TCH], F32, tag="acc", bufs=2,
                                        name="acc")
                        for nl in range(NNL):
                            nc.tensor.matmul(
                                accv[:],
                                ckv_sb[:, nl * T + tb * P:nl * T + (tb + 1) * P],
                                wuv_sb[:, nl * HLOC * HS:(nl + 1) * HLOC * HS],
                                start=(nl == 0),
                                stop=(nl == NNL - 1),
                            )
                        if i % 2 == 0:
                            nc.vector.tensor_copy(
                                vv[:, tb * TCH:(tb + 1) * TCH], accv[:])
                        else:
                            nc.scalar.copy(
                                vv[:, tb * TCH:(tb + 1) * TCH], accv[:])
                        acck = pwk.tile([P, TCH], F32,
                                        tag=("wo" if i % 2 else "dn"), bufs=1,
                                        name="wacc")
                        for pr2 in range(NNL // 2):
                            nc.tensor.matmul(
                                acck[:],
                                wukv[:, 2 * pr2:2 * pr2 + 2, h * P:(h + 1) * P],
                                ckv8v[:, 2 * pr2:2 * pr2 + 2, tsl],
                                start=(pr2 == 0),
                                stop=(pr2 == NNL // 2 - 1),
                                perf_mode=DR,
                            )
                        nc.scalar.activation(
                            kk8[h][:, tsl], acck[:], Copy, scale=1.0 / WUS)
                        rsl = slice(T + ch * TCH, T + (ch + 1) * TCH)
                        nc.gpsimd.tensor_scalar_mul(
                            kk8[h][0:RHD, rsl], kr[:, tsl], QKS)

                dnorm = pwk.tile([P, TCH], F32, tag="dn", bufs=1, name="dn")

                def emit_wo_tile(tq_prev, cs, ohs, cp_eng=0, tag="wo"):
                    qsl2 = slice(tq_prev * TCH, (tq_prev + 1) * TCH)
                    acc = pwk.tile([P, TCH], F32, tag=tag,
                                   bufs=(1 if tag == "wo" else 2), name="wacc")
                    for hh in range(HLOC):
                        nc.tensor.matmul(
                            acc[:],
                            wo_sb[:, hh * C + cs * P:hh * C + (cs + 1) * P],
                            ohs[hh][:],
                            start=(hh == 0),
                            stop=(hh == HLOC - 1),
                        )
                    ot = pat.tile([P, TCH], BF16, tag="ot", bufs=3, name="ot")
                    if cp_eng % 2 == 0:
                        nc.vector.tensor_copy(ot[:], acc[:])
                    else:
                        nc.scalar.copy(ot[:], acc[:])
                    nc.sync.dma_start(
                        out_ext.ap()[cs * P:(cs + 1) * P, qsl2],
                        ot[:],
                    )

                oh_prev = None
                pending_norm = [None]
                ghead = [0]

                # bf16 view of the dnorm bank: den_row [1, 512] lives at
                # bf16 cols [16:528] (den cols occupy f32 cols [0:8]).
                dnorm_bf = dnorm[:].bitcast(BF16)

                def make_norm(outU_, oh_t, base, npair_):
                    def run():
                        if DEN_TRICK:
                            den_sb = pat.tile([P, 4], BF16, tag="dsb", bufs=2,
                                              name="dsb")
                            nc.vector.tensor_copy(den_sb[:],
                                                  dnorm[:, base:base + 4])
                            for pr_ in range(1, npair_):
                                nc.vector.tensor_add(
                                    den_sb[:], den_sb[:],
                                    dnorm[:, base + 4 * pr_:
                                          base + 4 * pr_ + 4])
                            for qb in range(4):
                                nc.tensor.matmul(
                                    dnorm_bf[0:1,
                                             128 + qb * P:128 + (qb + 1) * P],
                                    den_sb[:, qb:qb + 1], ident[:],
                                    is_transpose=True,
                                    skip_group_check=True)
                            rsrc = dnorm_bf[0:1, 128:128 + TCH]
                        else:
                            rsrc = dnorm[0:1, 0:TCH]
                        recipb = pat.tile([1, TCH], BF16, tag="rcb",
                                          name="rcb", bufs=2)
                        with nc.allow_low_precision(reason="recip fits bf16"):
                            nc.vector.reciprocal(recipb[:], rsrc)
                        bcast = pat.tile([P, TCH], BF16, tag="bcs", bufs=2,
                                         name="bcs")
                        nc.gpsimd.partition_broadcast(bcast[:], recipb[:])
                        nc.vector.tensor_mul(oh_t[:], outU_[:], bcast[:])
                    return run

                for tq in range(NCH):
                    oh_cur = [pat.tile([P, TCH], BF16, tag=f"oh{i}",
                                       name=f"oh{i}", bufs=2)
                              for i in range(HLOC)]
                    pairs_total = HLOC * 2 * (tq + 1)
                    pairs_done = 0
                    wo_state = [0]

                    def wo_pace():
                        if tq == 0:
                            return
                        target = min(16, (pairs_done * 16 + pairs_total - 1)
                                     // pairs_total + 1)
                        while wo_state[0] < target:
                            emit_wo_tile(tq - 1, wo_state[0], oh_prev,
                                         cp_eng=0)
                            wo_state[0] += 1

                    for h in range(HLOC):
                        outU = pwk.tile([P, TCH], F32, tag="acc", bufs=2,
                                        name="acc")
                        kkv = kk8[h][:].rearrange("p (a t) -> p a t", a=2)
                        qkv = qk8[h][:].rearrange("p (a t) -> p a t", a=2)
                        npair = 2 * (tq + 1)
                        base = (ghead[0] % 2) * 32
                        ghead[0] += 1

                        # den accumulation chains: dnorm[:, base+qb] over
                        # k-tiles, probability tile as stationary, ones col as
                        # moving (1-row matmuls).  Every qb chain starts at
                        # kt=0 and ends at its diagonal tile kt = 4*tq + qb.
                        def emit_dp(p):
                            Ptp, kts, pr_ = p
                            for i, kt in enumerate(kts):
                                diag = kt // 4 == tq
                                ks = kt % 4
                                off = ks * P if diag else 0
                                nc.tensor.matmul(
                                    outU[:, off:],
                                    vv[:, kt * TCH + h * P:
                                       kt * TCH + (h + 1) * P],
                                    Ptp[:, i * TCH + off:(i + 1) * TCH],
                                    start=(kt == 0),
                                    stop=(kt == 4 * tq + 3),
                                    skip_group_check=True,
                                )
                            if not DEN_TRICK:
                                for i, kt in enumerate(kts):
                                    diag = kt // 4 == tq
                                    off = (kt % 4) * P if diag else 0
                                    nc.tensor.matmul(
                                        dnorm[0:1, off:TCH],
                                        ones[:, 0:1],
                                        Ptp[:, i * TCH + off:(i + 1) * TCH],
                                        start=(kt == 0),
                                        stop=(kt == 4 * tq + 3),
                                        skip_group_check=True,
                                    )
                                return
                            # per-(pair, qb) committed chains: at most one
                            # open accumulation chain per PSUM bank at any
                            # time (a start=True matmul wipes the bank's
                            # uncommitted accumulator state).
                            for qb in range(4):
                                col = base + pr_ * 4 + qb
                                valid = [(i, kt) for i, kt in enumerate(kts)
                                         if not (kt // 4 == tq
                                                 and qb < kt % 4)]
                                if not valid:
                                    nc.tensor.matmul(
                                        dnorm[:, col:col + 1],
                                        zers[:], ones[:, 0:1],
                                        start=True, stop=True,
                                        skip_group_check=True,
                                    )
                                    continue
                                for j, (i, kt) in enumerate(valid):
                                    nc.tensor.matmul(
                                        dnorm[:, col:col + 1],
                                        Ptp[:, i * TCH + qb * P:
                                            i * TCH + (qb + 1) * P],
                                        ones[:, 0:1],
                                        start=(j == 0),
                                        stop=(j == len(valid) - 1),
                                        skip_group_check=True,
                                    )

                        prev = None
                        for pr in range(npair):
                            kt0 = 2 * pr
                            kt1 = 2 * pr + 1
                            diag0 = kt0 // 4 == tq
                            diag1 = kt1 // 4 == tq
                            off0 = (kt0 % 4) * P if diag0 else 0
                            off1 = (kt1 % 4) * P if diag1 else 0
                            ST2 = pbig.tile([P, 2 * TCH], F32, tag="big",
                                            name="big")
                            nc.tensor.matmul(
                                ST2[:, off0:TCH],
                                kkv[:, :, kt0 * P:(kt0 + 1) * P],
                                qkv[:, :, tq * TCH + off0:(tq + 1) * TCH],
                                start=True, stop=True,
                                perf_mode=DR,
                            )
                            nc.tensor.matmul(
                                ST2[:, TCH + off1:2 * TCH],
                                kkv[:, :, kt1 * P:(kt1 + 1) * P],
                                qkv[:, :, tq * TCH + off1:(tq + 1) * TCH],
                                start=True, stop=True,
                                perf_mode=DR,
                            )
                            Pt = pat.tile([P, 2 * TCH], BF16, tag="pt",
                                          bufs=5, name="pt")
                            if diag0:
                                nc.scalar.activation(Pt[:, off0:TCH],
                                                     ST2[:, off0:TCH],
                                                     Exp, scale=SCALE8)
                                nc.scalar.activation(Pt[:, TCH + off1:],
                                                     ST2[:, TCH + off1:],
                                                     Exp, scale=SCALE8)
                                # causal boundary: multiplicative 0/1 mask on
                                # the diagonal 128-block, on the Pool engine
                                nc.gpsimd.tensor_mul(
                                    Pt[:, off0:off0 + P],
                                    Pt[:, off0:off0 + P], m2b[:])
                                nc.gpsimd.tensor_mul(
                                    Pt[:, TCH + off1:TCH + off1 + P],
                                    Pt[:, TCH + off1:TCH + off1 + P], m2b[:])
                            else:
                                nc.scalar.activation(Pt[:], ST2[:],
                                                     Exp, scale=SCALE8)
                            if pr == 1 and pending_norm[0] is not None:
                                pending_norm[0]()
                                pending_norm[0] = None
                            if prev is not None:
                                emit_dp(prev)
                                pairs_done += 1
                                wo_pace()
                            prev = (Pt, (kt0, kt1), pr)
                        emit_dp(prev)
                        pairs_done += 1
                        wo_pace()
                        pending_norm[0] = make_norm(outU, oh_cur[h], base, npair)
                    if tq + 1 < NCH:
                        qr_chunk(tq + 1)
                    oh_prev = oh_cur
                # flush the last head's normalization, then the final chunk's
                # W_o with copies alternating engines to drain fast
                if pending_norm[0] is not None:
                    pending_norm[0]()
                    pending_norm[0] = None
                for cs in range(C // P):
                    emit_wo_tile(NCH - 1, cs, oh_prev, cp_eng=cs % 2,
                                 tag=("wo" if cs % 2 else "acc"))

    nc.compile()
    return nc


def _get_nc():
    if "nc" not in _NC_CACHE:
        _NC_CACHE["nc"] = build()
    return _NC_CACHE["nc"]


def kernel(x, freqs_cos, freqs_sin, W_dq, W_uq, W_dkv, W_uk, W_uv, W_qr, W_kr,
           W_o, trace=False, **trace_kwargs):
    nc = _get_nc()
    bf = ml_dtypes.bfloat16
    f8 = ml_dtypes.float8_e4m3fn
    cT8 = lambda a: np.ascontiguousarray(
        (np.asarray(a, dtype=np.float32).T * WUS).astype(f8))
    f32 = lambda a: np.asarray(a, dtype=np.float32)
    cT = lambda a: np.ascontiguousarray(f32(a).T.astype(bf))

    x = f32(x)
    cos = f32(freqs_cos)
    sin = f32(freqs_sin)

    # host-side preprocessing (shared across cores)
    wdq8 = cT8(W_dq)                      # [C, NL] fp8
    wdkvT = cT(W_dkv)                     # [C, NL] bf16
    perm_r = np.concatenate([np.arange(0, RHD, 2), np.arange(1, RHD, 2)])
    wkr8 = cT8(f32(W_kr)[perm_r])         # [C, RHD] fp8 planar
    # rope tables: 4x-planar duplicated [128, T]
    caT = np.ascontiguousarray(
        np.tile(cos.T, (4, 1)).astype(bf))  # [128, T]
    saT = np.ascontiguousarray(
        np.tile(sin.T, (4, 1)).astype(bf))
    xTb = [np.ascontiguousarray(x[b].T.astype(bf)) for b in range(B)]
    x8b = [np.ascontiguousarray((x[b].T * SX).astype(f8)) for b in range(B)]

    W_qr_f = f32(W_qr)
    in_maps = []
    for c in range(8):
        b, r = divmod(c, 4)
        hsl = slice(r * HLOC * HS, (r + 1) * HLOC * HS)
        # W_qr rows per head pair: [h0 re(32) | h1 re(32) | h0 im(32) | h1 im(32)]
        wqr_rows = []
        for pr in range(HLOC // 2):
            h0 = r * HLOC * RHD + (2 * pr) * RHD
            h1 = r * HLOC * RHD + (2 * pr + 1) * RHD
            re0 = W_qr_f[h0:h0 + RHD][np.arange(0, RHD, 2)]
            re1 = W_qr_f[h1:h1 + RHD][np.arange(0, RHD, 2)]
            im0 = W_qr_f[h0:h0 + RHD][np.arange(1, RHD, 2)]
            im1 = W_qr_f[h1:h1 + RHD][np.arange(1, RHD, 2)]
            wqr_rows += [re0, re1, im0, im1]
        wqrT = np.ascontiguousarray(
            (np.concatenate(wqr_rows, axis=0).T * WUS).astype(f8))  # [NL, 256]
        in_maps.append({
            "xT": xTb[b],
            "x8": x8b[b],
            "xo8": np.ascontiguousarray(x8b[b][:, r * TCH:(r + 1) * TCH]),
            "wdq8": wdq8, "wdkvT": wdkvT, "wkr8": wkr8,
            "wuqT": cT8(f32(W_uq)[hsl]),
            "wukT": cT8(f32(W_uk)[hsl]),
            "wuvT": cT(f32(W_uv)[hsl]),
            "wqrT": wqrT,
            "woT": cT(f32(W_o)[:, hsl]),
            "caT": caT, "saT": saT,
        })
    res = run_bass_kernel_spmd(nc, in_maps, core_ids=list(range(8)),
                               trace=trace, **trace_kwargs)
    out = np.zeros((B, T, C), dtype=np.float32)
    for c in range(8):
        b = c // 4
        out[b] += res.results[c]["out"].astype(np.float32).T
    kernel.last_result = res
    return out


# revision 26
# speedup vs baseline: 1.0453x; 1.0292x over previous
"""MLA-style attention kernel for 8 TRN2 NeuronCores (v5).

Sharding: core c -> batch b = c//4, heads r*4..r*4+3 where r = c%4.

v5 vs v3-baseline: the AllGather is gone -- every core computes cq for
the FULL T with fp8 DoubleRow matmuls (same PE cost as the old own-chunk
bf16 cq, no collective, no dependency stall).  The score path is fully
fp8-DR (cq, kr, K up-projection contract in fp8 pairs); the value path
(ckv -> V -> PV -> W_o) stays bf16 end-to-end for accuracy.  The softmax
denominator is computed with 1-row matmuls (probability tile stationary,
a ones column moving), transposed to row form with one PE transpose, so
the old 512-row ones-matmuls disappear.  exp runs on [128,1024] score
pairs (two k-tiles per 2-bank PSUM tile) to halve ACT instruction count.
W_o tiles for the previous chunk are interleaved into the attention pair
loop so PE never waits on exp; output copies run on the Pool engine.
Rope multiplies run on 64-row two-head bands and the final sub/add write
fp8 score operands directly (no separate convert).  Q/V/K up-projection
chains are interleaved round-robin with their converts spread across
DVE/Pool/ACT so no single engine paces phase B.
"""
import math
import numpy as np
import ml_dtypes

import concourse.bass as bass
import concourse.bacc as bacc
import concourse.mybir as mybir
import concourse.tile as tile
from concourse.bass_utils import run_bass_kernel_spmd

F32 = mybir.dt.float32
BF16 = mybir.dt.bfloat16
FP8 = mybir.dt.float8e4
QKS = 16.0            # fp8 scale for q/k operands
WUS = 64.0            # fp8 scale for weights
SX = 4.0              # fp8 scale for x (score path)
Exp = mybir.ActivationFunctionType.Exp
Copy = mybir.ActivationFunctionType.Copy

B, T, C = 2, 2048, 2048
H = 16
HS = 128
NL = 512
RHD = 64
HLOC = 4              # heads per core
P = 128
NNL = NL // P         # 4 latent p-tiles
TCH = 512
NCH = T // TCH        # 4 T-chunks
NCT = C // P          # 16 contraction p-tiles over C
SCALE = 1.0 / math.sqrt(HS + RHD)
SCALE8 = SCALE / (QKS * QKS)
NEG = -1.0e30
DR = mybir.MatmulPerfMode.DoubleRow

_NC_CACHE = {}
DEN_TRICK = True


def build():
    nc = bacc.Bacc("TRN2", target_bir_lowering=False, debug=False, num_devices=8)

    xT_ext = nc.dram_tensor("xT", [C, T], BF16, kind="ExternalInput")
    x8_ext = nc.dram_tensor("x8", [C, T], FP8, kind="ExternalInput")
    wdq8_ext = nc.dram_tensor("wdq8", [C, NL], FP8, kind="ExternalInput")
    wdkvT_ext = nc.dram_tensor("wdkvT", [C, NL], BF16, kind="ExternalInput")
    wkr8_ext = nc.dram_tensor("wkr8", [C, RHD], FP8, kind="ExternalInput")
    wuqT_ext = nc.dram_tensor("wuqT", [NL, HLOC * HS], FP8, kind="ExternalInput")
    wukT_ext = nc.dram_tensor("wukT", [NL, HLOC * HS], FP8, kind="ExternalInput")
    wuvT_ext = nc.dram_tensor("wuvT", [NL, HLOC * HS], BF16, kind="ExternalInput")
    wqrT_ext = nc.dram_tensor("wqrT", [NL, HLOC * RHD], FP8, kind="ExternalInput")
    woT_ext = nc.dram_tensor("woT", [HLOC * HS, C], BF16, kind="ExternalInput")
    caT_ext = nc.dram_tensor("caT", [P, T], BF16, kind="ExternalInput")
    saT_ext = nc.dram_tensor("saT", [P, T], BF16, kind="ExternalInput")
    out_ext = nc.dram_tensor("out", [C, T], BF16, kind="ExternalOutput")
    xo8_ext = nc.dram_tensor("xo8", [C, TCH], FP8, kind="ExternalInput")
    agin_dram = nc.dram_tensor("agin", [NL, TCH], FP8)
    agout_dram = nc.dram_tensor("agout", [NCH, NL, TCH], FP8)

    ones_dram = nc.inline_tensor(np.ones((P, P), dtype=ml_dtypes.bfloat16),
                                 name="onesc")
    ident_dram = nc.inline_tensor(np.eye(P, dtype=ml_dtypes.bfloat16),
                                  name="identc")
    # boundary mask for the diagonal 128-col sub-block of S^T tiles [k, q]:
    # m2[jj, u] = 0 if u >= jj else -1e30
    m2 = np.zeros((P, P), dtype=ml_dtypes.bfloat16)
    for jj in range(P):
        m2[jj, jj:] = 1.0
    m2_dram = nc.inline_tensor(m2, name="m2c")

    with tile.TileContext(nc) as tc:
        with (
            tc.tile_pool(name="pers", bufs=1) as pers,
            tc.tile_pool(name="pbig", bufs=2, space="PSUM") as pbig,
            tc.tile_pool(name="pwk", bufs=1, space="PSUM") as pwk,
        ):
            ones = pers.tile([P, P], BF16, tag="ones", name="ones")
            zers = pers.tile([P, P], BF16, tag="zers", name="zers")
            nc.gpsimd.memset(zers[:], 0.0)
            ident = pers.tile([P, P], BF16, tag="ident", name="ident")
            m2b = pers.tile([P, P], BF16, tag="m2b", name="m2b")

            ca = pers.tile([P, T], BF16, tag="ca", name="ca")
            sa = pers.tile([P, T], BF16, tag="sa", name="sa")
            # PE p-state warmup: dummy matmuls on not-yet-loaded tiles keep the
            # tensor engine continuously busy through the initial DMA wait so
            # real matmuls start at full clock.
            for wi in range(14):
                warm = pbig.tile([P, 2 * TCH], F32, tag="big", name="big")
                nc.tensor.matmul(
                    warm[:, 0:TCH], ca[:, 0:P], sa[:, 0:TCH],
                    start=True, stop=True, skip_group_check=True,
                )

            # persistent activations
            ckv_sb = pers.tile([P, NNL * T], BF16, tag="ckv", name="ckv")
            ckv8 = pers.tile([P, NNL * T], FP8, tag="ckv8", name="ckv8")
            cq8 = pers.tile([P, NNL * T], FP8, tag="cq8", name="cq8")
            kr = pers.tile([RHD, T], BF16, tag="kr", name="kr")

            # score operands + V live in the pers pool so their memsets can
            # run at t=0 under the initial DMA window.
            qk8 = [pers.tile([P, 2 * T], FP8, tag=f"qk8{i}", name=f"qk8{i}")
                   for i in range(HLOC)]
            kk8 = [pers.tile([P, 2 * T], FP8, tag=f"kk8{i}", name=f"kk8{i}")
                   for i in range(HLOC)]
            for i in range(HLOC):
                nc.gpsimd.memset(qk8[i][RHD:P, T:2 * T], 0.0)
                nc.gpsimd.memset(kk8[i][RHD:P, T:2 * T], 0.0)
            vv = pers.tile([P, (T // P) * TCH], BF16, tag="vv", name="vv")

            # up/out-projection weights (preloaded early, used later)
            wuq_sb = pers.tile([P, NNL * HLOC * HS], FP8, tag="wuq", name="wuq")
            wuk_sb = pers.tile([P, NNL * HLOC * HS], FP8, tag="wuk", name="wuk")
            wuv_sb = pers.tile([P, NNL * HLOC * HS], BF16, tag="wuv", name="wuv")
            wqr_sb = pers.tile([P, NNL * HLOC * RHD], FP8, tag="wqr", name="wqr")
            wo_sb = pers.tile([P, HLOC * C], BF16, tag="wo", name="wo")

            # ---------------- phase A: down-projections ----------------
            with tc.tile_pool(name="pa", bufs=1) as pa:
                wdq8_sb = pa.tile([P, NCT * NL], FP8, tag="wdq", name="wdq")
                wdkv_sb = pa.tile([P, NCT * NL], BF16, tag="wdkv", name="wdkv")
                wkr8_sb = pa.tile([P, NCT * RHD], FP8, tag="wkr", name="wkr")
                xcs = [pa.tile([P, NCT * TCH], BF16, tag="xc", bufs=2,
                               name="xc") for _ in range(NCH)]
                x8s = [pa.tile([P, NCT * TCH], FP8, tag="x8c", bufs=2,
                               name="x8c") for _ in range(NCH)]

                def dma_x(ch):
                    tsl = slice(ch * TCH, (ch + 1) * TCH)
                    for st in range(4):
                        nc.sync.dma_start(
                            xcs[ch][:, st * 4 * TCH:(st + 1) * 4 * TCH].rearrange(
                                "p (a w) -> p a w", a=4),
                            xT_ext.ap()[st * 4 * P:(st + 1) * 4 * P, tsl].rearrange(
                                "(a p) w -> p a w", p=P),
                        )
                    for st in range(2):
                        nc.sync.dma_start(
                            x8s[ch][:, st * 8 * TCH:(st + 1) * 8 * TCH].rearrange(
                                "p (a w) -> p a w", a=8),
                            x8_ext.ap()[st * 8 * P:(st + 1) * 8 * P, tsl].rearrange(
                                "(a p) w -> p a w", p=P),
                        )

                # critical-path first: wdkv + chunk-0 x stripes interleaved
                for st in range(4):
                    nc.sync.dma_start(
                        wdkv_sb[:, st * 4 * NL:(st + 1) * 4 * NL].rearrange(
                            "p (a w) -> p a w", a=4),
                        wdkvT_ext.ap()[st * 4 * P:(st + 1) * 4 * P, :].rearrange(
                            "(a p) w -> p a w", p=P),
                    )
                    nc.sync.dma_start(
                        xcs[0][:, st * 4 * TCH:(st + 1) * 4 * TCH].rearrange(
                            "p (a w) -> p a w", a=4),
                        xT_ext.ap()[st * 4 * P:(st + 1) * 4 * P, 0:TCH].rearrange(
                            "(a p) w -> p a w", p=P),
                    )
                for st in range(2):
                    nc.sync.dma_start(
                        wdq8_sb[:, st * 8 * NL:(st + 1) * 8 * NL].rearrange(
                            "p (a w) -> p a w", a=8),
                        wdq8_ext.ap()[st * 8 * P:(st + 1) * 8 * P, :].rearrange(
                            "(a p) w -> p a w", p=P),
                    )
                    nc.sync.dma_start(
                        x8s[0][:, st * 8 * TCH:(st + 1) * 8 * TCH].rearrange(
                            "p (a w) -> p a w", a=8),
                        x8_ext.ap()[st * 8 * P:(st + 1) * 8 * P, 0:TCH].rearrange(
                            "(a p) w -> p a w", p=P),
                    )
                xo8 = pa.tile([P, NCT * TCH], FP8, tag="xo8", name="xo8")
                for st in range(2):
                    nc.sync.dma_start(
                        xo8[:, st * 8 * TCH:(st + 1) * 8 * TCH].rearrange(
                            "p (a w) -> p a w", a=8),
                        xo8_ext.ap()[st * 8 * P:(st + 1) * 8 * P, :].rearrange(
                            "(a p) w -> p a w", p=P),
                    )
                nc.sync.dma_start(
                    wkr8_sb[:].rearrange("p (a w) -> p a w", a=NCT),
                    wkr8_ext.ap().rearrange("(a p) w -> p a w", p=P),
                )
                dma_x(1)
                # non-critical loads on the Pool (SWDGE) queue
                nc.gpsimd.dma_start(out=ones[:], in_=ones_dram.ap())
                nc.gpsimd.dma_start(out=ident[:], in_=ident_dram.ap())
                nc.gpsimd.dma_start(out=m2b[:], in_=m2_dram.ap())
                nc.gpsimd.dma_start(out=ca[:], in_=caT_ext.ap())
                nc.gpsimd.dma_start(out=sa[:], in_=saT_ext.ap())
                # phase-B weights: needed only after ~70us, issue behind x
                for ext, sb in ((wuqT_ext, wuq_sb), (wqrT_ext, wqr_sb),
                                (wukT_ext, wuk_sb), (wuvT_ext, wuv_sb)):
                    nc.sync.dma_start(
                        sb[:].rearrange("p (a w) -> p a w", a=NNL),
                        ext.ap().rearrange("(a p) w -> p a w", p=P),
                    )
                nc.sync.dma_start(
                    wo_sb[:].rearrange("p (a w) -> p a w", a=HLOC),
                    woT_ext.ap().rearrange("(a p) w -> p a w", p=P),
                )

                wdq8v = wdq8_sb[:].rearrange("p (a w) -> p a w", a=NCT)
                wkr8v = wkr8_sb[:].rearrange("p (a w) -> p a w", a=NCT)

                # cq for the OWN T-chunk only (host stages xo8 per core);
                # AllGathered in fp8 across the 4-core group and consumed
                # directly by the fp8 Q up-projections.  The collective runs
                # under the whole of phase A.
                xo8v = xo8[:].rearrange("p (a w) -> p a w", a=NCT)
                cq_own = pa.tile([P, NNL * TCH], FP8, tag="cqo", name="cqo")
                for ot in range(NNL):
                    acc = pwk.tile([P, TCH], F32, tag="acc", bufs=2,
                                   name="acc")
                    for cp in range(NCT // 2):
                        nc.tensor.matmul(
                            acc[:],
                            wdq8v[:, 2 * cp:2 * cp + 2, ot * P:(ot + 1) * P],
                            xo8v[:, 2 * cp:2 * cp + 2, :],
                            start=(cp == 0),
                            stop=(cp == NCT // 2 - 1),
                            perf_mode=DR,
                        )
                    nc.scalar.activation(
                        cq_own[:, ot * TCH:(ot + 1) * TCH], acc[:],
                        Copy, scale=QKS / (WUS * SX),
                    )
                nc.sync.dma_start(
                    agin_dram.ap().rearrange("(a p) w -> p a w", p=P),
                    cq_own[:].rearrange("p (a w) -> p a w", a=NNL),
                )
                nc.gpsimd.collective_compute(
                    "AllGather",
                    mybir.AluOpType.bypass,
                    replica_groups=[[0, 1, 2, 3], [4, 5, 6, 7]],
                    ins=[agin_dram.ap().opt()],
                    outs=[agout_dram.ap().opt()],
                )
                for ch in range(NCH):
                    tsl = slice(ch * TCH, (ch + 1) * TCH)
                    if ch + 2 < NCH:
                        dma_x(ch + 2)
                    xcv = xcs[ch][:].rearrange("p (a w) -> p a w", a=NCT)
                    x8v = x8s[ch][:].rearrange("p (a w) -> p a w", a=NCT)

                    # ckv (bf16): ct-outer over 4 concurrent PSUM groups
                    accs = [pbig.tile([P, 2 * TCH], F32, tag="big", name="big")
                            for _ in range(2)]
                    for ct in range(NCT):
                        for ot in range(NNL):
                            nc.tensor.matmul(
                                accs[ot // 2][:, (ot % 2) * TCH:(ot % 2 + 1) * TCH],
                                wdkv_sb[:, ct * NL + ot * P:ct * NL + (ot + 1) * P],
                                xcv[:, ct, :],
                                start=(ct == 0),
                                stop=(ct == NCT - 1),
                            )
                    with nc.allow_low_precision(reason="fp8 latents"):
                        for ot in range(NNL):
                            src = accs[ot // 2][:, (ot % 2) * TCH:
                                                (ot % 2 + 1) * TCH]
                            dst = slice(ot * T + ch * TCH,
                                        ot * T + (ch + 1) * TCH)
                            if ot % 2 == 0:
                                nc.vector.tensor_copy(ckv_sb[:, dst], src)
                                nc.scalar.activation(ckv8[:, dst], src, Copy,
                                                     scale=QKS)
                            else:
                                nc.scalar.copy(ckv_sb[:, dst], src)
                                nc.vector.tensor_scalar_mul(ckv8[:, dst], src,
                                                            QKS)

                    # kr (fp8 DoubleRow), rope on DVE
                    acck = pwk.tile([P, TCH], F32, tag="wo", bufs=1,
                                    name="wacc")
                    for cp in range(NCT // 2):
                        nc.tensor.matmul(
                            acck[0:RHD, :],
                            wkr8v[:, 2 * cp:2 * cp + 2, 0:RHD],
                            x8v[:, 2 * cp:2 * cp + 2, :],
                            start=(cp == 0),
                            stop=(cp == NCT // 2 - 1),
                            perf_mode=DR,
                        )
                    krst = pa.tile([RHD, TCH], BF16, tag="krst", bufs=2,
                                   name="krst")
                    nc.scalar.activation(krst[:], acck[0:RHD, :], Copy,
                                         scale=1.0 / (WUS * SX))
                    tmp = pa.tile([RHD, TCH], BF16, tag="rtmp", bufs=2,
                                  name="rtmp")
                    # kr is a single 64-row head: 32-row bands
                    nc.vector.tensor_mul(tmp[0:32, :], krst[32:64, :], sa[32:64, tsl])
                    nc.vector.tensor_mul(tmp[32:64, :], krst[32:64, :], ca[32:64, tsl])
                    nc.vector.tensor_mul(kr[0:32, tsl], krst[0:32, :], ca[0:32, tsl])
                    nc.vector.tensor_mul(kr[32:64, tsl], krst[0:32, :], sa[0:32, tsl])
                    nc.vector.tensor_sub(kr[0:32, tsl], kr[0:32, tsl], tmp[0:32, :])
                    nc.vector.tensor_add(kr[32:64, tsl], kr[32:64, tsl], tmp[32:64, :])

                # gathered cq8 lands mid-phase-A; one rearranged DMA placed
                # after the chunk loop so the SP queue never stalls on the
                # collective
                for nl in range(NNL):
                    nc.sync.dma_start(
                        cq8[:, nl * T:(nl + 1) * T].rearrange(
                            "p (ch w) -> p ch w", ch=NCH),
                        agout_dram.ap()[:, nl * P:(nl + 1) * P, :].rearrange(
                            "ch p w -> p ch w"),
                    )

            # ------------- phase B/C: up-projections + attention -------------
            with (
                tc.tile_pool(name="ph", bufs=1) as ph,
                tc.tile_pool(name="pat", bufs=1) as pat,
            ):
                wuqv = wuq_sb[:].rearrange("p (a w) -> p a w", a=NNL)
                wukv = wuk_sb[:].rearrange("p (a w) -> p a w", a=NNL)
                wqrv = wqr_sb[:].rearrange("p (a w) -> p a w", a=NNL)
                cq8v = cq8[:].rearrange("p (a w) -> p a w", a=NNL)
                ckv8v = ckv8[:].rearrange("p (a w) -> p a w", a=NNL)

                # Q rope: packed 2 heads per matmul with host-planar layout
                # rows [0:64]=re(h0|h1), [64:128]=im(h0|h1).  The final
                # sub/add write the fp8 score operand directly (values are
                # pre-scaled by the qst copy).  Chunk ch is produced
                # just-in-time: ch0 up front, ch(tq+1) pipelined inside the
                # attention loop.
                def qr_chunk(ch):
                    Tsl = slice(T + ch * TCH, T + (ch + 1) * TCH)
                    for pr in range(HLOC // 2):
                        acc = pwk.tile([P, TCH], F32, tag="acc", bufs=2,
                                       name="acc")
                        for pr2 in range(NNL // 2):
                            nc.tensor.matmul(
                                acc[:],
                                wqrv[:, 2 * pr2:2 * pr2 + 2, pr * P:(pr + 1) * P],
                                cq8v[:, 2 * pr2:2 * pr2 + 2,
                                     ch * TCH:(ch + 1) * TCH],
                                start=(pr2 == 0),
                                stop=(pr2 == NNL // 2 - 1),
                                perf_mode=DR,
                            )
                        qst = pat.tile([P, TCH], BF16, tag="qst", bufs=2,
                                       name="qst")
                        # 1024*qr -> 16*qr so the rope output is fp8-ready
                        nc.scalar.activation(qst[:], acc[:], Copy,
                                             scale=1.0 / WUS)
                        tmp = pat.tile([P, TCH], BF16, tag="rtmp2", bufs=2,
                                       name="rtmp2")
                        # two-head 64-row multiplies, per-head 32-row fp8 writes
                        nc.vector.tensor_mul(tmp[0:64, :], qst[64:128, :],
                                             sa[64:128, ch * TCH:(ch + 1) * TCH])
                        nc.vector.tensor_mul(tmp[64:128, :], qst[64:128, :],
                                             ca[64:128, ch * TCH:(ch + 1) * TCH])
                        qre = pat.tile([P, TCH], BF16, tag="qre", bufs=2,
                                       name="qre")
                        nc.vector.tensor_mul(qre[0:64, :], qst[0:64, :],
                                             ca[0:64, ch * TCH:(ch + 1) * TCH])
                        nc.vector.tensor_mul(qre[64:128, :], qst[0:64, :],
                                             sa[0:64, ch * TCH:(ch + 1) * TCH])
                        with nc.allow_low_precision(reason="fp8 score operand"):
                            for sub in range(2):
                                h = pr * 2 + sub
                                ss = slice(sub * 32, sub * 32 + 32)
                                s2 = slice(64 + sub * 32, 64 + sub * 32 + 32)
                                nc.vector.tensor_sub(
                                    qk8[h][0:32, Tsl], qre[ss, :], tmp[ss, :])
                                nc.vector.tensor_add(
                                    qk8[h][32:64, Tsl], qre[s2, :], tmp[s2, :])

                qr_chunk(0)
                # interleave Q-content / V / K-content / K-rope so PE stays
                # fed and the converts spread across DVE / Pool / ACT.
                with nc.allow_low_precision(reason="fp8 score operand"):
                    for i in range(16):
                        h, ch = divmod(i, NCH)
                        tsl = slice(ch * TCH, (ch + 1) * TCH)
                        accq = pwk.tile([P, TCH], F32, tag="acc", bufs=2,
                                        name="acc")
                        for pr2 in range(NNL // 2):
                            nc.tensor.matmul(
                                accq[:],
                                wuqv[:, 2 * pr2:2 * pr2 + 2, h * P:(h + 1) * P],
                                cq8v[:, 2 * pr2:2 * pr2 + 2, tsl],
                                start=(pr2 == 0),
                                stop=(pr2 == NNL // 2 - 1),
                                perf_mode=DR,
                            )
                        nc.vector.tensor_scalar_mul(
                            qk8[h][:, tsl], accq[:], 1.0 / WUS)
                        # V (bf16) in natural [t, (head, hs)] layout
                        tb = i
                        accv = pwk.tile([P, TCH], F32, tag="acc", bufs=2,
                                        name="acc")
                        for nl in range(NNL):
                            nc.tensor.matmul(
                                accv[:],
                                ckv_sb[:, nl * T + tb * P:nl * T + (tb + 1) * P],
                                wuv_sb[:, nl * HLOC * HS:(nl + 1) * HLOC * HS],
                                start=(nl == 0),
                                stop=(nl == NNL - 1),
                            )
                        if i % 2 == 0:
                            nc.vector.tensor_copy(
                                vv[:, tb * TCH:(tb + 1) * TCH], accv[:])
                        else:
                            nc.scalar.copy(
                                vv[:, tb * TCH:(tb + 1) * TCH], accv[:])
                        acck = pwk.tile([P, TCH], F32,
                                        tag=("wo" if i % 2 else "dn"), bufs=1,
                                        name="wacc")
                        for pr2 in range(NNL // 2):
                            nc.tensor.matmul(
                                acck[:],
                                wukv[:, 2 * pr2:2 * pr2 + 2, h * P:(h + 1) * P],
                                ckv8v[:, 2 * pr2:2 * pr2 + 2, tsl],
                                start=(pr2 == 0),
                                stop=(pr2 == NNL // 2 - 1),
                                perf_mode=DR,
                            )
                        nc.scalar.activation(
                            kk8[h][:, tsl], acck[:], Copy, scale=1.0 / WUS)
                        rsl = slice(T + ch * TCH, T + (ch + 1) * TCH)
                        nc.gpsimd.tensor_scalar_mul(
                            kk8[h][0:RHD, rsl], kr[:, tsl], QKS)

                dnorm = pwk.tile([P, TCH], F32, tag="dn", bufs=1, name="dn")

                def emit_wo_tile(tq_prev, cs, ohs, cp_eng=0, tag="wo"):
                    qsl2 = slice(tq_prev * TCH, (tq_prev + 1) * TCH)
                    acc = pwk.tile([P, TCH], F32, tag=tag,
                                   bufs=(1 if tag == "wo" else 2), name="wacc")
                    for hh in range(HLOC):
                        nc.tensor.matmul(
                            acc[:],
                            wo_sb[:, hh * C + cs * P:hh * C + (cs + 1) * P],
                            ohs[hh][:],
                            start=(hh == 0),
                            stop=(hh == HLOC - 1),
                        )
                    ot = pat.tile([P, TCH], BF16, tag="ot", bufs=3, name="ot")
                    if cp_eng % 2 == 0:
                        nc.vector.tensor_copy(ot[:], acc[:])
                    else:
                        nc.scalar.copy(ot[:], acc[:])
                    nc.sync.dma_start(
                        out_ext.ap()[cs * P:(cs + 1) * P, qsl2],
                        ot[:],
                    )

                oh_prev = None
                pending_norm = [None]
                ghead = [0]

                # bf16 view of the dnorm bank: den_row [1, 512] lives at
                # bf16 cols [16:528] (den cols occupy f32 cols [0:8]).
                dnorm_bf = dnorm[:].bitcast(BF16)

                def make_norm(outU_, oh_t, base, npair_):
                    def run():
                        if DEN_TRICK:
                            den_sb = pat.tile([P, 4], BF16, tag="dsb", bufs=2,
                                              name="dsb")
                            nc.vector.tensor_copy(den_sb[:],
                                                  dnorm[:, base:base + 4])
                            for pr_ in range(1, npair_):
                                nc.vector.tensor_add(
                                    den_sb[:], den_sb[:],
                                    dnorm[:, base + 4 * pr_:
                                          base + 4 * pr_ + 4])
                            for qb in range(4):
                                nc.tensor.matmul(
                                    dnorm_bf[0:1,
                                             128 + qb * P:128 + (qb + 1) * P],
                                    den_sb[:, qb:qb + 1], ident[:],
                                    is_transpose=True,
                                    skip_group_check=True)
                            rsrc = dnorm_bf[0:1, 128:128 + TCH]
                        else:
                            rsrc = dnorm[0:1, 0:TCH]
                        recipb = pat.tile([1, TCH], BF16, tag="rcb",
                                          name="rcb", bufs=2)
                        with nc.allow_low_precision(reason="recip fits bf16"):
                            nc.vector.reciprocal(recipb[:], rsrc)
                        bcast = pat.tile([P, TCH], BF16, tag="bcs", bufs=2,
                                         name="bcs")
                        nc.gpsimd.partition_broadcast(bcast[:], recipb[:])
                        nc.vector.tensor_mul(oh_t[:], outU_[:], bcast[:])
                    return run

                for tq in range(NCH):
                    oh_cur = [pat.tile([P, TCH], BF16, tag=f"oh{i}",
                                       name=f"oh{i}", bufs=2)
                              for i in range(HLOC)]
                    pairs_total = HLOC * 2 * (tq + 1)
                    pairs_done = 0
                    wo_state = [0]

                    def wo_pace():
                        if tq == 0:
                            return
                        target = min(16, (pairs_done * 16 + pairs_total - 1)
                                     // pairs_total + 1)
                        while wo_state[0] < target:
                            emit_wo_tile(tq - 1, wo_state[0], oh_prev,
                                         cp_eng=0)
                            wo_state[0] += 1

                    for h in range(HLOC):
                        outU = pwk.tile([P, TCH], F32, tag="acc", bufs=2,
                                        name="acc")
                        kkv = kk8[h][:].rearrange("p (a t) -> p a t", a=2)
                        qkv = qk8[h][:].rearrange("p (a t) -> p a t", a=2)
                        npair = 2 * (tq + 1)
                        base = (ghead[0] % 2) * 32
                        ghead[0] += 1

                        # den accumulation chains: dnorm[:, base+qb] over
                        # k-tiles, probability tile as stationary, ones col as
                        # moving (1-row matmuls).  Every qb chain starts at
                        # kt=0 and ends at its diagonal tile kt = 4*tq + qb.
                        def emit_dp(p):
                            Ptp, kts, pr_ = p
                            for i, kt in enumerate(kts):
                                diag = kt // 4 == tq
                                ks = kt % 4
                                off = ks * P if diag else 0
                                nc.tensor.matmul(
                                    outU[:, off:],
                                    vv[:, kt * TCH + h * P:
                                       kt * TCH + (h + 1) * P],
                                    Ptp[:, i * TCH + off:(i + 1) * TCH],
                                    start=(kt == 0),
                                    stop=(kt == 4 * tq + 3),
                                    skip_group_check=True,
                                )
                            if not DEN_TRICK:
                                for i, kt in enumerate(kts):
                                    diag = kt // 4 == tq
                                    off = (kt % 4) * P if diag else 0
                                    nc.tensor.matmul(
                                        dnorm[0:1, off:TCH],
                                        ones[:, 0:1],
                                        Ptp[:, i * TCH + off:(i + 1) * TCH],
                                        start=(kt == 0),
                                        stop=(kt == 4 * tq + 3),
                                        skip_group_check=True,
                                    )
                                return
                            # per-(pair, qb) committed chains: at most one
                            # open accumulation chain per PSUM bank at any
                            # time (a start=True matmul wipes the bank's
                            # uncommitted accumulator state).
                            for qb in range(4):
                                col = base + pr_ * 4 + qb
                                valid = [(i, kt) for i, kt in enumerate(kts)
                                         if not (kt // 4 == tq
                                                 and qb < kt % 4)]
                                if not valid:
                                    nc.tensor.matmul(
                                        dnorm[:, col:col + 1],
                                        zers[:], ones[:, 0:1],
                                        start=True, stop=True,
                                        skip_group_check=True,
                                    )
                                    continue
                                for j, (i, kt) in enumerate(valid):
                                    nc.tensor.matmul(
                                        dnorm[:, col:col + 1],
                                        Ptp[:, i * TCH + qb * P:
                                            i * TCH + (qb + 1) * P],
                                        ones[:, 0:1],
                                        start=(j == 0),
                                        stop=(j == len(valid) - 1),
                                        skip_group_check=True,
                                    )

                        prev = None
                        for pr in range(npair):
                            kt0 = 2 * pr
                            kt1 = 2 * pr + 1
                            diag0 = kt0 // 4 == tq
                            diag1 = kt1 // 4 == tq
                            off0 = (kt0 % 4) * P if diag0 else 0
                            off1 = (kt1 % 4) * P if diag1 else 0
                            ST2 = pbig.tile([P, 2 * TCH], F32, tag="big",
                                            name="big")
                            nc.tensor.matmul(
                                ST2[:, off0:TCH],
                                kkv[:, :, kt0 * P:(kt0 + 1) * P],
                                qkv[:, :, tq * TCH + off0:(tq + 1) * TCH],
                                start=True, stop=True,
                                perf_mode=DR,
                            )
                            nc.tensor.matmul(
                                ST2[:, TCH + off1:2 * TCH],
                                kkv[:, :, kt1 * P:(kt1 + 1) * P],
                                qkv[:, :, tq * TCH + off1:(tq + 1) * TCH],
                                start=True, stop=True,
                                perf_mode=DR,
                            )
                            Pt = pat.tile([P, 2 * TCH], BF16, tag="pt",
                                          bufs=5, name="pt")
                            if diag0:
                                nc.scalar.activation(Pt[:, off0:TCH],
                                                     ST2[:, off0:TCH],
                                                     Exp, scale=SCALE8)
                                nc.scalar.activation(Pt[:, TCH + off1:],
                                                     ST2[:, TCH + off1:],
                                                     Exp, scale=SCALE8)
                                # causal boundary: multiplicative 0/1 mask on
                                # the diagonal 128-block, on the Pool engine
                                nc.gpsimd.tensor_mul(
                                    Pt[:, off0:off0 + P],
                                    Pt[:, off0:off0 + P], m2b[:])
                                nc.gpsimd.tensor_mul(
                                    Pt[:, TCH + off1:TCH + off1 + P],
                                    Pt[:, TCH + off1:TCH + off1 + P], m2b[:])
                            else:
                                nc.scalar.activation(Pt[:], ST2[:],
                                                     Exp, scale=SCALE8)
                            if pr == 1 and pending_norm[0] is not None:
                                pending_norm[0]()
                                pending_norm[0] = None
                            if prev is not None:
                                emit_dp(prev)
                                pairs_done += 1
                                wo_pace()
                            prev = (Pt, (kt0, kt1), pr)
                        emit_dp(prev)
                        pairs_done += 1
                        wo_pace()
                        pending_norm[0] = make_norm(outU, oh_cur[h], base, npair)
                    if tq + 1 < NCH:
                        qr_chunk(tq + 1)
                    oh_prev = oh_cur
                # flush the last head's normalization, then the final chunk's
                # W_o with copies alternating engines to drain fast
                if pending_norm[0] is not None:
                    pending_norm[0]()
                    pending_norm[0] = None
                for cs in range(C // P):
                    emit_wo_tile(NCH - 1, cs, oh_prev, cp_eng=cs % 2,
                                 tag=("wo" if cs % 2 else "acc"))

    nc.compile()
    return nc


def _get_nc():
    if "nc" not in _NC_CACHE:
        _NC_CACHE["nc"] = build()
    return _NC_CACHE["nc"]


def kernel(x, freqs_cos, freqs_sin, W_dq, W_uq, W_dkv, W_uk, W_uv, W_qr, W_kr,
           W_o, trace=False, **trace_kwargs):
    nc = _get_nc()
    bf = ml_dtypes.bfloat16
    f8 = ml_dtypes.float8_e4m3fn
    cT8 = lambda a: np.ascontiguousarray(
        (np.asarray(a, dtype=np.float32).T * WUS).astype(f8))
    f32 = lambda a: np.asarray(a, dtype=np.float32)
    cT = lambda a: np.ascontiguousarray(f32(a).T.astype(bf))

    x = f32(x)
    cos = f32(freqs_cos)
    sin = f32(freqs_sin)

    # host-side preprocessing (shared across cores)
    wdq8 = cT8(W_dq)                      # [C, NL] fp8
    wdkvT = cT(W_dkv)                     # [C, NL] bf16
    perm_r = np.concatenate([np.arange(0, RHD, 2), np.arange(1, RHD, 2)])
    wkr8 = cT8(f32(W_kr)[perm_r])         # [C, RHD] fp8 planar
    # rope tables: 4x-planar duplicated [128, T]
    caT = np.ascontiguousarray(
        np.tile(cos.T, (4, 1)).astype(bf))  # [128, T]
    saT = np.ascontiguousarray(
        np.tile(sin.T, (4, 1)).astype(bf))
    xTb = [np.ascontiguousarray(x[b].T.astype(bf)) for b in range(B)]
    x8b = [np.ascontiguousarray((x[b].T * SX).astype(f8)) for b in range(B)]

    W_qr_f = f32(W_qr)
    in_maps = []
    for c in range(8):
        b, r = divmod(c, 4)
        hsl = slice(r * HLOC * HS, (r + 1) * HLOC * HS)
        # W_qr rows per head pair: [h0 re(32) | h1 re(32) | h0 im(32) | h1 im(32)]
        wqr_rows = []
        for pr in range(HLOC // 2):
            h0 = r * HLOC * RHD + (2 * pr) * RHD
            h1 = r * HLOC * RHD + (2 * pr + 1) * RHD
            re0 = W_qr_f[h0:h0 + RHD][np.arange(0, RHD, 2)]
            re1 = W_qr_f[h1:h1 + RHD][np.arange(0, RHD, 2)]
            im0 = W_qr_f[h0:h0 + RHD][np.arange(1, RHD, 2)]
            im1 = W_qr_f[h1:h1 + RHD][np.arange(1, RHD, 2)]
            wqr_rows += [re0, re1, im0, im1]
        wqrT = np.ascontiguousarray(
            (np.concatenate(wqr_rows, axis=0).T * WUS).astype(f8))  # [NL, 256]
        in_maps.append({
            "xT": xTb[b],
            "x8": x8b[b],
            "xo8": np.ascontiguousarray(x8b[b][:, r * TCH:(r + 1) * TCH]),
            "wdq8": wdq8, "wdkvT": wdkvT, "wkr8": wkr8,
            "wuqT": cT8(f32(W_uq)[hsl]),
            "wukT": cT8(f32(W_uk)[hsl]),
            "wuvT": cT(f32(W_uv)[hsl]),
            "wqrT": wqrT,
            "woT": cT(f32(W_o)[:, hsl]),
            "caT": caT, "saT": saT,
        })
    res = run_bass_kernel_spmd(nc, in_maps, core_ids=list(range(8)),
                               trace=trace, **trace_kwargs)
    out = np.zeros((B, T, C), dtype=np.float32)
    for c in range(8):
        b = c // 4
        out[b] += res.results[c]["out"].astype(np.float32).T
    kernel.last_result = res
    return out
